# revision 37
# baseline (speedup 1.0000x reference)
"""Trainium2 Bass kernel for nn_Bottleneck_CSA_ConvBlock.

Computation (per image, C=64, H=W=160):
    y  = silu(bn1(conv3x3(x, w1)))
    fq = conv3x3(y, wq); fk = conv3x3(y, wk); fv = conv3x3(y, wv)
    k_sum = fk.sum(ch, h); f_scores[c] = scale * sum_hw fq[c,h,w]*k_sum[w]
    scores = softmax_c(f_scores)
    out = x + relu(bn2(scores*fv + y))

Key idea vs the plain lowering: conv contraction is only C=64 (half the
128-row PE array). Each image is stored twice in SBUF -- channels
unshifted on one partition half, shifted by one padded row (+WP) on the
other -- so a single 128-row matmul computes TWO dy-taps at once: the 9
taps of a 3x3 conv become 3 pair-matmuls plus 3 dy=2 singles (2/3 the
matmul instructions; measured ~200ns fixed cost per matmul makes count
matter as much as streamed cycles). Blocks are nr=3 rows (N=480, one
PSUM bank). The two images run on the two 64-column PE strips; emission
is software-pipelined (block b's img1 interleaves with block b-1's img2)
so the strips overlap while both images share one PSUM bank per block
(img2's group re-opens the bank with skip_group_check -- HW start only
clears has_written bits, img1's finished data is untouched).

HW constraints discovered (mini_mm.py probes): a PSUM bank's
accumulation group must keep ONE region and ONE row-position; mixing
row positions (e.g. (0,0) then (64,0)) wedges the device. Column-
disjoint same-row matmul streams overlap; diagonal ones did not.

Layout (DRAM-staged x and on-chip y):
    region A: partitions 0-63  = img1 unshifted ; 64-127 = img1 shifted(+WP)
    region B: partitions 0-63  = img2 shifted   ; 64-127 = img2 unshifted
so conv outputs land lane-aligned ([img1|img2] on psum partitions).
Pass-1 BN+SiLU runs full-width into a scratch tile; four SBUF->SBUF
DMAs distribute it to the four y quarter-layouts (unshifted + shifted),
and column sums reduce from the scratch at full width.

f_scores/k_sum fold to linear functionals of y's column sums (fq/fk
never materialized). Residual x and the output travel as bf16.

Sharding: pure data parallelism, 2 images per core across 8 cores.
"""

import numpy as np

C = 64
H = W = 160
HP = WP = 162          # padded
IMG = H * W            # 25600
PIMG = HP * WP         # 26244
LP = PIMG + 60         # region stride in the paired buffers
NCH = 26               # x chunk rows
CHH = NCH * WP + 4     # chunk region stride (+slack for dx=2 edge views)
GRP = 4                # blocks per scratch/distribute super
BLOCKS = [(1 + 3 * i, 3) for i in range(53)] + [(160, 1)]
BN_EPS = 1e-5

_CACHED = {}


def _build_nc(loop=0, act1=None):
    import concourse.bass as bass
    import concourse.tile as tile
    from concourse import bacc, mybir
    from concourse.masks import make_identity

    dt = mybir.dt
    AF = mybir.ActivationFunctionType
    AX = mybir.AxisListType
    ACT1 = AF.Silu if act1 is None else getattr(AF, act1)
    f32 = dt.float32
    bf16 = dt.bfloat16

    nc = bacc.Bacc("TRN2", target_bir_lowering=False, debug=False, num_devices=8)

    xp_d = nc.dram_tensor("xp", [128, 2 * LP], bf16, kind="ExternalInput")
    xr_d = nc.dram_tensor("xr", [128, IMG], bf16, kind="ExternalInput")
    w1p1_d = nc.dram_tensor("w1p1", [128, 3, 64], bf16, kind="ExternalInput")
    w1p2_d = nc.dram_tensor("w1p2", [128, 3, 64], bf16, kind="ExternalInput")
    w1s_d = nc.dram_tensor("w1s", [128, 3, 64], bf16, kind="ExternalInput")
    wvp1_d = nc.dram_tensor("wvp1", [128, 3, 64], bf16, kind="ExternalInput")
    wvp2_d = nc.dram_tensor("wvp2", [128, 3, 64], bf16, kind="ExternalInput")
    wvs_d = nc.dram_tensor("wvs", [128, 3, 64], bf16, kind="ExternalInput")
    wq_d = nc.dram_tensor("wqt", [128, 9, 65], bf16, kind="ExternalInput")
    bn1s_d = nc.dram_tensor("bn1s", [128, 1], f32, kind="ExternalInput")
    bn1b_d = nc.dram_tensor("bn1b", [128, 1], f32, kind="ExternalInput")
    bn2s_d = nc.dram_tensor("bn2s", [128, 1], f32, kind="ExternalInput")
    bn2b_d = nc.dram_tensor("bn2b", [128, 1], f32, kind="ExternalInput")
    out_d = nc.dram_tensor("out", [128, IMG], bf16, kind="ExternalOutput")

    A = 0
    B = LP

    def r3(ap):
        return ap.rearrange("p (r c) -> p r c", c=WP)

    with tile.TileContext(nc) as tc:
        ctx_lp = nc.allow_low_precision("bf16 matmul path; fp32 PSUM accumulation")
        ctx_lp.__enter__()

        def mk_mms(src, wpt, wst, bX, img, r0, nr, ps):
            """Thunk list: 3 pair-MMs + 3 single-MMs for one image/block.

            img=0: strip (0,0), psum [0:64], pairs low=dy0/high=dy1 at
            base row r0-1, singles via unshifted low at base r0+1.
            img=1: strip (0,64), psum [64:128], pairs (swapped weights) at
            r0-1, singles via shifted low half at base r0.
            img2's matmuls re-open the bank's accumulation group after
            img1's closed it -- legal on HW (start only clears has_written
            bits, img1's data is untouched); skip_group_check silences the
            sim's partition-blind group tracker.
            """
            po = ps[0:64] if img == 0 else ps[64:128]
            tp = (0, 0) if img == 0 else (0, 64)
            skip = img == 1
            th = []
            for dx in range(3):
                o = bX(r0 - 1, dx)
                th.append(lambda o=o, dx=dx: nc.tensor.matmul(
                    po[:, 0:nr * W], wpt[:, dx, :],
                    r3(src[0:128, o:o + nr * WP])[:, :, 0:W],
                    start=(dx == 0), stop=False, tile_position=tp,
                    skip_group_check=skip))
            srow = r0 + 1 if img == 0 else r0
            for dx in range(3):
                o = bX(srow, dx)
                th.append(lambda o=o, dx=dx: nc.tensor.matmul(
                    po[:, 0:nr * W], wst[0:64, dx, :],
                    r3(src[0:64, o:o + nr * WP])[:, :, 0:W],
                    start=False, stop=(dx == 2), tile_position=tp,
                    skip_group_check=skip))
            return th

        def conv_pass(block_iter):
            """Software-pipelined emission: block b's img1 MMs interleave
            with block b-1's img2 MMs (disjoint PE column strips overlap).
            block_iter yields (mk1, mk2, epilogue) per block."""
            prev2 = prev_epi = None
            for mk1, mk2, epi in block_iter:
                cur1 = mk1()
                if prev2 is None:
                    for f in cur1:
                        f()
                else:
                    for a, b in zip(prev2, cur1):
                        a()
                        b()
                    prev_epi()
                prev2, prev_epi = mk2(), epi
            for f in prev2:
                f()
            prev_epi()

        def body():
            with (
                tc.tile_pool(name="const", bufs=1) as const,
                tc.tile_pool(name="ybuf", bufs=1) as ybuf,
                tc.tile_pool(name="small", bufs=1) as small,
            ):
                wts = {}
                for nm, d in (("w1p1", w1p1_d), ("w1p2", w1p2_d), ("w1s", w1s_d),
                              ("wvp1", wvp1_d), ("wvp2", wvp2_d), ("wvs", wvs_d)):
                    t = const.tile([128, 3, 64], bf16, name="w_" + nm)
                    nc.scalar.dma_start(out=t[:], in_=d.ap())
                    wts[nm] = t
                wq_sb = const.tile([128, 9, 65], bf16)
                nc.scalar.dma_start(out=wq_sb[:], in_=wq_d.ap())
                bn1s = const.tile([128, 1], f32)
                nc.scalar.dma_start(out=bn1s[:], in_=bn1s_d.ap())
                bn1b = const.tile([128, 1], f32)
                nc.scalar.dma_start(out=bn1b[:], in_=bn1b_d.ap())
                bn2s = const.tile([128, 1], f32)
                nc.scalar.dma_start(out=bn2s[:], in_=bn2s_d.ap())
                bn2b = const.tile([128, 1], f32)
                nc.scalar.dma_start(out=bn2b[:], in_=bn2b_d.ap())
                ident = const.tile([128, 128], f32)
                make_identity(nc, ident[:])
                ones_sb = const.tile([128, 64], bf16)
                nc.vector.memset(ones_sb[:], 1.0)

                # persistent paired y (bf16)
                y = ybuf.tile([128, 2 * LP], bf16)
                # zero pads on the unshifted halves: top+bottom rows, l/r cols
                for p0, p1, reg in ((0, 64, A), (64, 128, B)):
                    nc.vector.memset(y[p0:p1, reg:reg + WP], 0.0)
                    nc.vector.memset(
                        y[p0:p1, reg + (H + 1) * WP:reg + (H + 2) * WP], 0.0)
                    v3 = r3(y[p0:p1, reg:reg + PIMG])
                    nc.vector.memset(v3[:, 1:H + 1, 0:1], 0.0)
                    nc.vector.memset(v3[:, 1:H + 1, WP - 1:WP], 0.0)
                # shifted halves: address row 160 = image row 161 = zeros
                for p0, p1, reg in ((64, 128, A), (0, 64, B)):
                    nc.vector.memset(y[p0:p1, reg + H * WP:reg + (H + 1) * WP], 0.0)

                C_sb = small.tile([128, WP], bf16)
                CmL = small.tile([128, WP], bf16)
                CmF = small.tile([128, WP], bf16)
                part = small.tile([128, WP], f32, tag="part")
                q0s = small.tile([65, 160], bf16)
                q1s = small.tile([65, 160], bf16)
                t0s = small.tile([64, 160], f32)
                t1s = small.tile([64, 160], f32)
                fs0 = small.tile([64, 1], f32)
                fs1 = small.tile([64, 1], f32)
                frow = small.tile([1, 128], f32)
                srow = small.tile([1, 128], f32)
                mx = small.tile([1, 1], f32, tag="mx")
                sm = small.tile([1, 1], f32, tag="sm")
                rs = small.tile([1, 1], f32, tag="rs")
                scores = small.tile([128, 1], f32)
                # pass-1 scratch: full-width ACT target, distributed to the
                # paired y regions by DMA. Col pads pre-zeroed; ACT only
                # writes interiors, so pads stay zero across reuses.
                sp0 = small.tile([128, 12 * WP], bf16, tag="sp0")
                sp1 = small.tile([128, 12 * WP], bf16, tag="sp1")
                for sp in (sp0, sp1):
                    v = sp.rearrange("p (r c) -> p r c", c=WP)
                    nc.vector.memset(v[:, :, 0:1], 0.0)
                    nc.vector.memset(v[:, :, WP - 1:WP], 0.0)

                xpap = xp_d.ap()

                # ---------------- pass 1: conv1 -> y (paired), column sums ----
                with (
                    tc.tile_pool(name="chunks", bufs=2) as chunks,
                    tc.tile_pool(name="ps1", bufs=8, space="PSUM") as ps1,
                ):
                    cur = {}
                    nblk = len(BLOCKS)

                    def p1_iter():
                        for bi, (r0, nr) in enumerate(BLOCKS):
                            k = bi // 8
                            si = bi // GRP
                            sp = (sp0, sp1)[si % 2]
                            sup_r0 = BLOCKS[si * GRP][0]
                            sup_last = (bi % GRP == GRP - 1) or bi == nblk - 1

                            ps = ps1.tile([128, 512], f32, tag="ps",
                                          name="ps1t")

                            def mk1(k=k, first=(bi % 8 == 0), r0=r0, nr=nr,
                                    ps=ps):
                                if first:
                                    ir0 = 24 * k
                                    nir = 26 if k < 6 else 18
                                    ch = chunks.tile([128, 2 * CHH], bf16,
                                                     tag="ch", name="cht")
                                    pieces = ((0, 14), (14, nir - 14)) \
                                        if k == 0 else ((0, nir),)
                                    for po, pn in pieces:
                                        nc.sync.dma_start(
                                            out=ch[:, po * WP:(po + pn) * WP],
                                            in_=xpap[:, A + (ir0 + po) * WP:
                                                     A + (ir0 + po + pn) * WP])
                                        nc.sync.dma_start(
                                            out=ch[:, CHH + po * WP:
                                                    CHH + (po + pn) * WP],
                                            in_=xpap[:, B + (ir0 + po) * WP:
                                                     B + (ir0 + po + pn) * WP])
                                    cur["ch"], cur["ir0"] = ch, ir0
                                ch, ir0 = cur["ch"], cur["ir0"]

                                def bA(row, dx):
                                    return (row - ir0) * WP + dx
                                return mk_mms(ch, wts["w1p1"], wts["w1s"],
                                              bA, 0, r0, nr, ps)

                            def mk2(r0=r0, nr=nr, ps=ps):
                                ch, ir0 = cur["ch"], cur["ir0"]

                                def bB(row, dx):
                                    return CHH + (row - ir0) * WP + dx
                                return mk_mms(ch, wts["w1p2"], wts["w1s"],
                                              bB, 1, r0, nr, ps)

                            def epi(bi=bi, r0=r0, nr=nr, ps=ps, sp=sp,
                                    sup_r0=sup_r0, sup_last=sup_last, si=si):
                                loff = (r0 - sup_r0) * WP
                                nc.scalar.activation(
                                    out=r3(sp[:, loff:loff + nr * WP])[:, :, 1:1 + W],
                                    in_=ps[0:128, 0:nr * W],
                                    func=ACT1, bias=bn1b[:], scale=bn1s[:])
                                if sup_last:
                                    srows = (r0 + nr) - sup_r0
                                    sl = srows * WP
                                    nc.sync.dma_start(
                                        out=y[0:64, A + sup_r0 * WP:
                                              A + sup_r0 * WP + sl],
                                        in_=sp[0:64, 0:sl])
                                    nc.sync.dma_start(
                                        out=y[64:128, B + sup_r0 * WP:
                                              B + sup_r0 * WP + sl],
                                        in_=sp[64:128, 0:sl])
                                    nc.gpsimd.dma_start(
                                        out=y[64:128, A + (sup_r0 - 1) * WP:
                                              A + (sup_r0 - 1) * WP + sl],
                                        in_=sp[0:64, 0:sl])
                                    nc.gpsimd.dma_start(
                                        out=y[0:64, B + (sup_r0 - 1) * WP:
                                              B + (sup_r0 - 1) * WP + sl],
                                        in_=sp[64:128, 0:sl])
                                    nc.vector.reduce_sum(
                                        part[:],
                                        r3(sp[:, 0:sl])
                                        .rearrange("p r c -> p c r"),
                                        axis=AX.X)
                                    if si == 0:
                                        nc.vector.tensor_copy(C_sb[:], part[:])
                                    else:
                                        nc.vector.tensor_add(C_sb[:], C_sb[:],
                                                             part[:])
                            yield mk1, mk2, epi

                    conv_pass(p1_iter())

                # ---------------- scores (small path) ----------------
                with tc.tile_pool(name="pss", bufs=2, space="PSUM") as pss:
                    nc.vector.tensor_sub(CmL[0:64, :], C_sb[0:64, :],
                                         y[0:64, A + H * WP:A + (H + 1) * WP])
                    nc.vector.tensor_sub(CmL[64:128, :], C_sb[64:128, :],
                                         y[64:128, B + H * WP:B + (H + 1) * WP])
                    nc.vector.tensor_sub(CmF[0:64, :], C_sb[0:64, :],
                                         y[0:64, A + WP:A + 2 * WP])
                    nc.vector.tensor_sub(CmF[64:128, :], C_sb[64:128, :],
                                         y[64:128, B + WP:B + 2 * WP])
                    s_of = {0: CmL, 1: C_sb, 2: CmF}

                    qp0 = pss.tile([65, 160], f32, tag="qp")
                    qp1 = pss.tile([65, 160], f32, tag="qp")
                    for k9 in range(9):
                        dy, dx = divmod(k9, 3)
                        src = s_of[dy]
                        nc.tensor.matmul(
                            qp0[:, :], wq_sb[0:64, k9, :], src[0:64, dx:dx + 160],
                            start=(k9 == 0), stop=(k9 == 8), tile_position=(0, 0))
                    for k9 in range(9):
                        dy, dx = divmod(k9, 3)
                        src = s_of[dy]
                        nc.tensor.matmul(
                            qp1[:, :], wq_sb[64:128, k9, :], src[64:128, dx:dx + 160],
                            start=(k9 == 0), stop=(k9 == 8), tile_position=(64, 0))
                    nc.vector.tensor_copy(q0s[:], qp0[:])
                    nc.vector.tensor_copy(q1s[:], qp1[:])

                    # broadcast k_sum row (partition 64) across 64 partitions
                    bc0 = pss.tile([64, 160], f32, tag="bc")
                    bc1 = pss.tile([64, 160], f32, tag="bc")
                    nc.tensor.matmul(bc0[:, :], ones_sb[64:65, :], q0s[64:65, :],
                                     start=True, stop=True, tile_position=(64, 0))
                    nc.tensor.matmul(bc1[:, :], ones_sb[64:65, :], q1s[64:65, :],
                                     start=True, stop=True, tile_position=(64, 0))
                    nc.vector.tensor_mul(t0s[:], q0s[0:64, :], bc0[:])
                    nc.vector.tensor_mul(t1s[:], q1s[0:64, :], bc1[:])
                    nc.vector.reduce_sum(fs0[:], t0s[:], axis=AX.X)
                    nc.vector.reduce_sum(fs1[:], t1s[:], axis=AX.X)

                    tr0 = pss.tile([1, 64], f32, tag="tr")
                    tr1 = pss.tile([1, 64], f32, tag="tr")
                    nc.tensor.transpose(tr0[:], fs0[:], ident[0:64, 0:64])
                    nc.tensor.transpose(tr1[:], fs1[:], ident[0:64, 0:64])
                    nc.vector.tensor_copy(frow[0:1, 0:64], tr0[:])
                    nc.vector.tensor_copy(frow[0:1, 64:128], tr1[:])

                    for img in range(2):
                        seg = frow[0:1, 64 * img:64 * img + 64]
                        oseg = srow[0:1, 64 * img:64 * img + 64]
                        nc.vector.reduce_max(mx[:], seg, axis=AX.X, negate=True)
                        nc.scalar.activation(out=oseg, in_=seg, func=AF.Exp,
                                             bias=mx[:], scale=1.0)
                        nc.vector.reduce_sum(sm[:], oseg, axis=AX.X)
                        nc.vector.reciprocal(rs[:], sm[:])
                        nc.vector.tensor_scalar_mul(oseg, oseg, rs[:])

                    psc = pss.tile([128, 1], f32, tag="psc")
                    nc.tensor.transpose(psc[:], srow[:], ident[0:1, 0:1])
                    nc.vector.tensor_copy(scores[:], psc[:])

                # ---------------- pass 2: conv_v -> epilogue -> out ----------
                with (
                    tc.tile_pool(name="ps2", bufs=4, space="PSUM") as ps2,
                    tc.tile_pool(name="epi", bufs=3) as epi_p,
                    tc.tile_pool(name="gio", bufs=2) as gio,
                ):
                    MUL = mybir.AluOpType.mult
                    ADD = mybir.AluOpType.add
                    cur2 = {}
                    nblk = len(BLOCKS)

                    def gA(row, dx):
                        return A + row * WP + dx

                    def gB(row, dx):
                        return B + row * WP + dx

                    def rw(ap, nr):
                        return ap.rearrange("p (r c) -> p r c", c=W)

                    def p2_iter():
                        xt = ot = None
                        for bi, (r0, nr) in enumerate(BLOCKS):
                            g = bi // 8
                            goff = 24 * g * W
                            glen = 3840 if g < 6 else 2560
                            grp_last = (bi % 8 == 7) or bi == nblk - 1

                            pp = ps2 if (bi < 4 or bi % 2 == 0 or
                                         pool2["alt"] is None) else pool2["alt"]
                            ps = pp.tile([128, 512], f32, tag="ps",
                                         name="ps2t")
                            if bi % 8 == 0:
                                xt = gio.tile([128, 3840], bf16, tag="xt",
                                              name="xt")
                                ot = gio.tile([128, 3840], bf16, tag="ot",
                                              name="ot")

                            def mk1(first=(bi % 8 == 0), r0=r0, nr=nr, ps=ps,
                                    goff=goff, glen=glen, xt=xt):
                                if first:
                                    nc.sync.dma_start(
                                        out=xt[:, 0:glen],
                                        in_=xr_d.ap()[:, goff:goff + glen])
                                return mk_mms(y, wts["wvp1"], wts["wvs"],
                                              gA, 0, r0, nr, ps)

                            def mk2(r0=r0, nr=nr, ps=ps):
                                return mk_mms(y, wts["wvp2"], wts["wvs"],
                                              gB, 1, r0, nr, ps)

                            def epi(r0=r0, nr=nr, ps=ps, goff=goff,
                                    glen=glen, grp_last=grp_last,
                                    xt=xt, ot=ot):
                                boff = (r0 - 1) * W - goff
                                M = nr * W
                                u2 = epi_p.tile([128, 3 * W], bf16, tag="u2")
                                nc.vector.scalar_tensor_tensor(
                                    rw(u2[0:64, 0:M], nr),
                                    rw(ps[0:64, 0:M], nr), scores[0:64],
                                    r3(y[0:64, A + r0 * WP:
                                         A + (r0 + nr) * WP])[:, :, 1:1 + W],
                                    MUL, ADD)
                                u = epi_p.tile([128, 3 * W], bf16, tag="u")
                                nc.scalar.mul(u[64:128, 0:M],
                                              ps[64:128, 0:M], scores[64:128])
                                nc.vector.tensor_add(
                                    rw(u2[64:128, 0:M], nr),
                                    rw(u[64:128, 0:M], nr),
                                    r3(y[64:128, B + r0 * WP:
                                         B + (r0 + nr) * WP])[:, :, 1:1 + W])
                                rt = epi_p.tile([128, 3 * W], bf16, tag="rt")
                                nc.scalar.activation(out=rt[:, 0:M],
                                                     in_=u2[:, 0:M],
                                                     func=AF.Relu,
                                                     bias=bn2b[:],
                                                     scale=bn2s[:])
                                nc.vector.tensor_add(ot[:, boff:boff + M],
                                                     rt[:, 0:M],
                                                     xt[:, boff:boff + M])
                                if goff == 23040 and boff + M == 1440:
                                    # last group: flush in two halves so the
                                    # final DMA tail is one small transfer
                                    nc.sync.dma_start(
                                        out=out_d.ap()[:, goff:goff + 1440],
                                        in_=ot[:, 0:1440])
                                elif goff == 23040 and boff + M == 2560:
                                    nc.sync.dma_start(
                                        out=out_d.ap()[:, goff + 1440:
                                                       goff + 2560],
                                        in_=ot[:, 1440:2560])
                                elif grp_last:
                                    nc.sync.dma_start(
                                        out=out_d.ap()[:, goff:goff + glen],
                                        in_=ot[:, 0:glen])
                            yield mk1, mk2, epi

                    conv_pass(p2_iter())

        if loop:
            with tc.For_i(0, loop, 1):
                body()
        else:
            body()
        ctx_lp.__exit__(None, None, None)
    nc.compile()
    return nc


def _get_nc():
    if "nc" not in _CACHED:
        _CACHED["nc"] = _build_nc()
    return _CACHED["nc"]


def _prep_weights(w_cv1, wq, wk, wv, g1, b1, m1, v1, g2, b2, m2, v2):
    import ml_dtypes
    bf = ml_dtypes.bfloat16

    def parts(w):
        # w [cout, cin, ky, kx] -> t [cin, ky, kx, cout]
        t = np.ascontiguousarray(w.transpose(1, 2, 3, 0))
        p1 = np.concatenate([t[:, 0], t[:, 1]], axis=0)   # low=dy0, high=dy1
        p2 = np.concatenate([t[:, 1], t[:, 0]], axis=0)   # low=dy1, high=dy0
        s = np.concatenate([t[:, 2], t[:, 2]], axis=0)    # dy2 duplicated
        return (np.ascontiguousarray(p1.astype(bf)),
                np.ascontiguousarray(p2.astype(bf)),
                np.ascontiguousarray(s.astype(bf)))

    w1p1, w1p2, w1s = parts(w_cv1)
    wvp1, wvp2, wvs = parts(wv)

    scale = 1.0 / (float(W) ** 0.5 * float(H) * float(H))
    q = wq.transpose(1, 2, 3, 0).reshape(C, 9, C) * scale    # [j, 9, c]
    ks = wk.sum(axis=0).reshape(C, 9, 1)                     # [j, 9, 1]
    qa = np.concatenate([q, ks], axis=2)                     # [j, 9, 65]
    wqt = np.ascontiguousarray(np.concatenate([qa, qa], axis=0).astype(bf))

    s1 = (g1 / np.sqrt(v1 + BN_EPS)).astype(np.float32)
    b1p = (b1 - m1 * s1).astype(np.float32)
    s2 = (g2 / np.sqrt(v2 + BN_EPS)).astype(np.float32)
    b2p = (b2 - m2 * s2).astype(np.float32)

    def dup(v):
        return np.ascontiguousarray(
            np.concatenate([v, v]).reshape(128, 1).astype(np.float32))

    return dict(w1p1=w1p1, w1p2=w1p2, w1s=w1s, wvp1=wvp1, wvp2=wvp2, wvs=wvs,
                wqt=wqt, bn1s=dup(s1), bn1b=dup(b1p),
                bn2s=dup(s2), bn2b=dup(b2p))


def _stage_x(x2):
    """x2: [2, C, H, W] f32 -> (xp [128, 2*LP] bf16, xr [128, IMG] bf16)."""
    import ml_dtypes
    bf = ml_dtypes.bfloat16
    xpad = np.zeros((2, C, HP, WP), np.float32)
    xpad[:, :, 1:1 + H, 1:1 + W] = x2
    flat = xpad.reshape(2, C, PIMG)
    sh = np.zeros_like(flat)
    sh[:, :, :PIMG - WP] = flat[:, :, WP:]
    xp = np.zeros((128, 2 * LP), bf)
    xp[0:64, 0:PIMG] = flat[0]
    xp[64:128, 0:PIMG] = sh[0]
    xp[0:64, B0:B0 + PIMG] = sh[1]
    xp[64:128, B0:B0 + PIMG] = flat[1]
    xr = np.ascontiguousarray(
        x2.reshape(2, C, IMG).reshape(128, IMG).astype(bf))
    return np.ascontiguousarray(xp), xr


B0 = LP


def _ensure_axon_devices():
    """Make sure jax can see the 8 axon-tunneled NeuronCores even if the
    calling process pinned JAX_PLATFORMS=cpu before importing us."""
    import os
    envp = os.environ.get("JAX_PLATFORMS", "")
    if envp and "axon" not in envp:
        os.environ.pop("JAX_PLATFORMS", None)
    import jax
    try:
        devs = jax.devices()
        if len(devs) >= 8 and all("cpu" not in str(d).lower() for d in devs[:8]):
            return
    except Exception:
        pass
    try:
        from jax._src import xla_bridge
        xla_bridge.backends.cache_clear()
    except Exception:
        pass
    try:
        import jax.extend.backend as jeb
        jeb.clear_backends()
    except Exception:
        pass


def kernel(x, w_cv1, g1, b1, m1, v1, wq, wk, wv, g2, b2, m2, v2):
    _ensure_axon_devices()
    from concourse.bass_utils import run_bass_kernel_spmd

    x = np.asarray(x, dtype=np.float32)
    consts = _prep_weights(
        np.asarray(w_cv1, np.float32), np.asarray(wq, np.float32),
        np.asarray(wk, np.float32), np.asarray(wv, np.float32),
        np.asarray(g1, np.float32), np.asarray(b1, np.float32),
        np.asarray(m1, np.float32), np.asarray(v1, np.float32),
        np.asarray(g2, np.float32), np.asarray(b2, np.float32),
        np.asarray(m2, np.float32), np.asarray(v2, np.float32))
    nc = _get_nc()
    in_maps = []
    for i in range(8):
        xp, xr = _stage_x(x[2 * i:2 * i + 2])
        m = {"xp": xp, "xr": xr}
        m.update(consts)
        in_maps.append(m)
    res = run_bass_kernel_spmd(nc, in_maps, core_ids=list(range(8)))
    outs = [np.asarray(r["out"]).astype(np.float32).reshape(2, C, H, W)
            for r in res.results]
    return np.concatenate(outs, axis=0)


# revision 38
# speedup vs baseline: 1.0026x; 1.0026x over previous
"""Trainium2 Bass kernel for nn_Bottleneck_CSA_ConvBlock.

Computation (per image, C=64, H=W=160):
    y  = silu(bn1(conv3x3(x, w1)))
    fq = conv3x3(y, wq); fk = conv3x3(y, wk); fv = conv3x3(y, wv)
    k_sum = fk.sum(ch, h); f_scores[c] = scale * sum_hw fq[c,h,w]*k_sum[w]
    scores = softmax_c(f_scores)
    out = x + relu(bn2(scores*fv + y))

Key idea vs the plain lowering: conv contraction is only C=64 (half the
128-row PE array). Each image is stored twice in SBUF -- channels
unshifted on one partition half, shifted by one padded row (+WP) on the
other -- so a single 128-row matmul computes TWO dy-taps at once: the 9
taps of a 3x3 conv become 3 pair-matmuls plus 3 dy=2 singles (2/3 the
matmul instructions; measured ~200ns fixed cost per matmul makes count
matter as much as streamed cycles). Blocks are nr=3 rows (N=480, one
PSUM bank). The two images run on the two 64-column PE strips; emission
is software-pipelined (block b's img1 interleaves with block b-1's img2)
so the strips overlap while both images share one PSUM bank per block
(img2's group re-opens the bank with skip_group_check -- HW start only
clears has_written bits, img1's finished data is untouched).

HW constraints discovered (mini_mm.py probes): a PSUM bank's
accumulation group must keep ONE region and ONE row-position; mixing
row positions (e.g. (0,0) then (64,0)) wedges the device. Column-
disjoint same-row matmul streams overlap; diagonal ones did not.

Layout (DRAM-staged x and on-chip y):
    region A: partitions 0-63  = img1 unshifted ; 64-127 = img1 shifted(+WP)
    region B: partitions 0-63  = img2 shifted   ; 64-127 = img2 unshifted
so conv outputs land lane-aligned ([img1|img2] on psum partitions).
Pass-1 BN+SiLU runs full-width into a scratch tile; four SBUF->SBUF
DMAs distribute it to the four y quarter-layouts (unshifted + shifted),
and column sums reduce from the scratch at full width.

f_scores/k_sum fold to linear functionals of y's column sums (fq/fk
never materialized). Residual x and the output travel as bf16.

Sharding: pure data parallelism, 2 images per core across 8 cores.
"""

import numpy as np

C = 64
H = W = 160
HP = WP = 162          # padded
IMG = H * W            # 25600
PIMG = HP * WP         # 26244
LP = PIMG + 60         # region stride in the paired buffers
NCH = 26               # x chunk rows
CHH = NCH * WP + 4     # chunk region stride (+slack for dx=2 edge views)
GRP = 4                # blocks per scratch/distribute super
BLOCKS = [(1 + 3 * i, 3) for i in range(53)] + [(160, 1)]
BN_EPS = 1e-5

_CACHED = {}


def _build_nc(loop=0, act1=None):
    import concourse.bass as bass
    import concourse.tile as tile
    from concourse import bacc, mybir
    from concourse.masks import make_identity

    dt = mybir.dt
    AF = mybir.ActivationFunctionType
    AX = mybir.AxisListType
    ACT1 = AF.Silu if act1 is None else getattr(AF, act1)
    f32 = dt.float32
    bf16 = dt.bfloat16

    nc = bacc.Bacc("TRN2", target_bir_lowering=False, debug=False, num_devices=8)

    xp_d = nc.dram_tensor("xp", [128, 2 * LP], bf16, kind="ExternalInput")
    xr_d = nc.dram_tensor("xr", [128, IMG], bf16, kind="ExternalInput")
    w1p1_d = nc.dram_tensor("w1p1", [128, 3, 64], bf16, kind="ExternalInput")
    w1p2_d = nc.dram_tensor("w1p2", [128, 3, 64], bf16, kind="ExternalInput")
    w1s_d = nc.dram_tensor("w1s", [128, 3, 64], bf16, kind="ExternalInput")
    wvp1_d = nc.dram_tensor("wvp1", [128, 3, 64], bf16, kind="ExternalInput")
    wvp2_d = nc.dram_tensor("wvp2", [128, 3, 64], bf16, kind="ExternalInput")
    wvs_d = nc.dram_tensor("wvs", [128, 3, 64], bf16, kind="ExternalInput")
    wq_d = nc.dram_tensor("wqt", [128, 9, 65], bf16, kind="ExternalInput")
    bn1s_d = nc.dram_tensor("bn1s", [128, 1], f32, kind="ExternalInput")
    bn1b_d = nc.dram_tensor("bn1b", [128, 1], f32, kind="ExternalInput")
    bn2s_d = nc.dram_tensor("bn2s", [128, 1], f32, kind="ExternalInput")
    bn2b_d = nc.dram_tensor("bn2b", [128, 1], f32, kind="ExternalInput")
    out_d = nc.dram_tensor("out", [128, IMG], bf16, kind="ExternalOutput")

    A = 0
    B = LP

    def r3(ap):
        return ap.rearrange("p (r c) -> p r c", c=WP)

    with tile.TileContext(nc) as tc:
        ctx_lp = nc.allow_low_precision("bf16 matmul path; fp32 PSUM accumulation")
        ctx_lp.__enter__()

        def mk_mms(src, wpt, wst, bX, img, r0, nr, ps):
            """Thunk list: 3 pair-MMs + 3 single-MMs for one image/block.

            img=0: strip (0,0), psum [0:64], pairs low=dy0/high=dy1 at
            base row r0-1, singles via unshifted low at base r0+1.
            img=1: strip (0,64), psum [64:128], pairs (swapped weights) at
            r0-1, singles via shifted low half at base r0.
            img2's matmuls re-open the bank's accumulation group after
            img1's closed it -- legal on HW (start only clears has_written
            bits, img1's data is untouched); skip_group_check silences the
            sim's partition-blind group tracker.
            """
            po = ps[0:64] if img == 0 else ps[64:128]
            tp = (0, 0) if img == 0 else (0, 64)
            skip = img == 1
            th = []
            for dx in range(3):
                o = bX(r0 - 1, dx)
                th.append(lambda o=o, dx=dx: nc.tensor.matmul(
                    po[:, 0:nr * W], wpt[:, dx, :],
                    r3(src[0:128, o:o + nr * WP])[:, :, 0:W],
                    start=(dx == 0), stop=False, tile_position=tp,
                    skip_group_check=skip))
            srow = r0 + 1 if img == 0 else r0
            for dx in range(3):
                o = bX(srow, dx)
                th.append(lambda o=o, dx=dx: nc.tensor.matmul(
                    po[:, 0:nr * W], wst[0:64, dx, :],
                    r3(src[0:64, o:o + nr * WP])[:, :, 0:W],
                    start=False, stop=(dx == 2), tile_position=tp,
                    skip_group_check=skip))
            return th

        def conv_pass(block_iter):
            """Software-pipelined emission: block b's img1 MMs interleave
            with block b-1's img2 MMs (disjoint PE column strips overlap).
            block_iter yields (mk1, mk2, epilogue) per block."""
            prev2 = prev_epi = None
            for mk1, mk2, epi in block_iter:
                cur1 = mk1()
                if prev2 is None:
                    for f in cur1:
                        f()
                else:
                    for a, b in zip(prev2, cur1):
                        a()
                        b()
                    prev_epi()
                prev2, prev_epi = mk2(), epi
            for f in prev2:
                f()
            prev_epi()

        def body():
            with (
                tc.tile_pool(name="const", bufs=1) as const,
                tc.tile_pool(name="ybuf", bufs=1) as ybuf,
                tc.tile_pool(name="small", bufs=1) as small,
            ):
                wts = {}
                for nm, d in (("w1p1", w1p1_d), ("w1p2", w1p2_d), ("w1s", w1s_d),
                              ("wvp1", wvp1_d), ("wvp2", wvp2_d), ("wvs", wvs_d)):
                    t = const.tile([128, 3, 64], bf16, name="w_" + nm)
                    nc.scalar.dma_start(out=t[:], in_=d.ap())
                    wts[nm] = t
                wq_sb = const.tile([128, 9, 65], bf16)
                nc.scalar.dma_start(out=wq_sb[:], in_=wq_d.ap())
                bn1s = const.tile([128, 1], f32)
                nc.scalar.dma_start(out=bn1s[:], in_=bn1s_d.ap())
                bn1b = const.tile([128, 1], f32)
                nc.scalar.dma_start(out=bn1b[:], in_=bn1b_d.ap())
                bn2s = const.tile([128, 1], f32)
                nc.scalar.dma_start(out=bn2s[:], in_=bn2s_d.ap())
                bn2b = const.tile([128, 1], f32)
                nc.scalar.dma_start(out=bn2b[:], in_=bn2b_d.ap())
                ident = const.tile([128, 128], f32)
                make_identity(nc, ident[:])
                ones_sb = const.tile([128, 64], bf16)
                nc.vector.memset(ones_sb[:], 1.0)

                # persistent paired y (bf16)
                y = ybuf.tile([128, 2 * LP], bf16)
                # zero pads on the unshifted halves: top+bottom rows, l/r cols
                for p0, p1, reg in ((0, 64, A), (64, 128, B)):
                    nc.vector.memset(y[p0:p1, reg:reg + WP], 0.0)
                    nc.vector.memset(
                        y[p0:p1, reg + (H + 1) * WP:reg + (H + 2) * WP], 0.0)
                    v3 = r3(y[p0:p1, reg:reg + PIMG])
                    nc.vector.memset(v3[:, 1:H + 1, 0:1], 0.0)
                    nc.vector.memset(v3[:, 1:H + 1, WP - 1:WP], 0.0)
                # shifted halves: address row 160 = image row 161 = zeros
                for p0, p1, reg in ((64, 128, A), (0, 64, B)):
                    nc.vector.memset(y[p0:p1, reg + H * WP:reg + (H + 1) * WP], 0.0)

                C_sb = small.tile([128, WP], bf16)
                CmL = small.tile([128, WP], bf16)
                CmF = small.tile([128, WP], bf16)
                part = small.tile([128, WP], f32, tag="part")
                q0s = small.tile([65, 160], bf16)
                q1s = small.tile([65, 160], bf16)
                t0s = small.tile([64, 160], f32)
                t1s = small.tile([64, 160], f32)
                fs0 = small.tile([64, 1], f32)
                fs1 = small.tile([64, 1], f32)
                frow = small.tile([1, 128], f32)
                srow = small.tile([1, 128], f32)
                mx = small.tile([1, 1], f32, tag="mx")
                sm = small.tile([1, 1], f32, tag="sm")
                rs = small.tile([1, 1], f32, tag="rs")
                scores = small.tile([128, 1], f32)
                # pass-1 scratch: full-width ACT target, distributed to the
                # paired y regions by DMA. Col pads pre-zeroed; ACT only
                # writes interiors, so pads stay zero across reuses.
                sp0 = small.tile([128, 12 * WP], bf16, tag="sp0")
                sp1 = small.tile([128, 12 * WP], bf16, tag="sp1")
                for sp in (sp0, sp1):
                    v = sp.rearrange("p (r c) -> p r c", c=WP)
                    nc.vector.memset(v[:, :, 0:1], 0.0)
                    nc.vector.memset(v[:, :, WP - 1:WP], 0.0)

                xpap = xp_d.ap()

                # ---------------- pass 1: conv1 -> y (paired), column sums ----
                with (
                    tc.tile_pool(name="chunks", bufs=2) as chunks,
                    tc.tile_pool(name="ps1", bufs=8, space="PSUM") as ps1,
                ):
                    cur = {}
                    nblk = len(BLOCKS)

                    def p1_iter():
                        for bi, (r0, nr) in enumerate(BLOCKS):
                            k = bi // 8
                            si = bi // GRP
                            sp = (sp0, sp1)[si % 2]
                            sup_r0 = BLOCKS[si * GRP][0]
                            sup_last = (bi % GRP == GRP - 1) or bi == nblk - 1

                            ps = ps1.tile([128, 512], f32, tag="ps",
                                          name="ps1t")

                            def mk1(k=k, first=(bi % 8 == 0), r0=r0, nr=nr,
                                    ps=ps):
                                if first:
                                    ir0 = 24 * k
                                    nir = 26 if k < 6 else 18
                                    ch = chunks.tile([128, 2 * CHH], bf16,
                                                     tag="ch", name="cht")
                                    pieces = ((0, 5), (5, 9), (14, nir - 14)) \
                                        if k == 0 else ((0, nir),)
                                    for po, pn in pieces:
                                        nc.sync.dma_start(
                                            out=ch[:, po * WP:(po + pn) * WP],
                                            in_=xpap[:, A + (ir0 + po) * WP:
                                                     A + (ir0 + po + pn) * WP])
                                        nc.sync.dma_start(
                                            out=ch[:, CHH + po * WP:
                                                    CHH + (po + pn) * WP],
                                            in_=xpap[:, B + (ir0 + po) * WP:
                                                     B + (ir0 + po + pn) * WP])
                                    cur["ch"], cur["ir0"] = ch, ir0
                                ch, ir0 = cur["ch"], cur["ir0"]

                                def bA(row, dx):
                                    return (row - ir0) * WP + dx
                                return mk_mms(ch, wts["w1p1"], wts["w1s"],
                                              bA, 0, r0, nr, ps)

                            def mk2(r0=r0, nr=nr, ps=ps):
                                ch, ir0 = cur["ch"], cur["ir0"]

                                def bB(row, dx):
                                    return CHH + (row - ir0) * WP + dx
                                return mk_mms(ch, wts["w1p2"], wts["w1s"],
                                              bB, 1, r0, nr, ps)

                            def epi(bi=bi, r0=r0, nr=nr, ps=ps, sp=sp,
                                    sup_r0=sup_r0, sup_last=sup_last, si=si):
                                loff = (r0 - sup_r0) * WP
                                nc.scalar.activation(
                                    out=r3(sp[:, loff:loff + nr * WP])[:, :, 1:1 + W],
                                    in_=ps[0:128, 0:nr * W],
                                    func=ACT1, bias=bn1b[:], scale=bn1s[:])
                                if sup_last:
                                    srows = (r0 + nr) - sup_r0
                                    sl = srows * WP
                                    nc.sync.dma_start(
                                        out=y[0:64, A + sup_r0 * WP:
                                              A + sup_r0 * WP + sl],
                                        in_=sp[0:64, 0:sl])
                                    nc.sync.dma_start(
                                        out=y[64:128, B + sup_r0 * WP:
                                              B + sup_r0 * WP + sl],
                                        in_=sp[64:128, 0:sl])
                                    nc.gpsimd.dma_start(
                                        out=y[64:128, A + (sup_r0 - 1) * WP:
                                              A + (sup_r0 - 1) * WP + sl],
                                        in_=sp[0:64, 0:sl])
                                    nc.gpsimd.dma_start(
                                        out=y[0:64, B + (sup_r0 - 1) * WP:
                                              B + (sup_r0 - 1) * WP + sl],
                                        in_=sp[64:128, 0:sl])
                                    nc.vector.reduce_sum(
                                        part[:],
                                        r3(sp[:, 0:sl])
                                        .rearrange("p r c -> p c r"),
                                        axis=AX.X)
                                    if si == 0:
                                        nc.vector.tensor_copy(C_sb[:], part[:])
                                    else:
                                        nc.vector.tensor_add(C_sb[:], C_sb[:],
                                                             part[:])
                            yield mk1, mk2, epi

                    conv_pass(p1_iter())

                # ---------------- scores (small path) ----------------
                with tc.tile_pool(name="pss", bufs=2, space="PSUM") as pss:
                    nc.vector.tensor_sub(CmL[0:64, :], C_sb[0:64, :],
                                         y[0:64, A + H * WP:A + (H + 1) * WP])
                    nc.vector.tensor_sub(CmL[64:128, :], C_sb[64:128, :],
                                         y[64:128, B + H * WP:B + (H + 1) * WP])
                    nc.vector.tensor_sub(CmF[0:64, :], C_sb[0:64, :],
                                         y[0:64, A + WP:A + 2 * WP])
                    nc.vector.tensor_sub(CmF[64:128, :], C_sb[64:128, :],
                                         y[64:128, B + WP:B + 2 * WP])
                    s_of = {0: CmL, 1: C_sb, 2: CmF}

                    qp0 = pss.tile([65, 160], f32, tag="qp")
                    qp1 = pss.tile([65, 160], f32, tag="qp")
                    for k9 in range(9):
                        dy, dx = divmod(k9, 3)
                        src = s_of[dy]
                        nc.tensor.matmul(
                            qp0[:, :], wq_sb[0:64, k9, :], src[0:64, dx:dx + 160],
                            start=(k9 == 0), stop=(k9 == 8), tile_position=(0, 0))
                    for k9 in range(9):
                        dy, dx = divmod(k9, 3)
                        src = s_of[dy]
                        nc.tensor.matmul(
                            qp1[:, :], wq_sb[64:128, k9, :], src[64:128, dx:dx + 160],
                            start=(k9 == 0), stop=(k9 == 8), tile_position=(64, 0))
                    nc.vector.tensor_copy(q0s[:], qp0[:])
                    nc.vector.tensor_copy(q1s[:], qp1[:])

                    # broadcast k_sum row (partition 64) across 64 partitions
                    bc0 = pss.tile([64, 160], f32, tag="bc")
                    bc1 = pss.tile([64, 160], f32, tag="bc")
                    nc.tensor.matmul(bc0[:, :], ones_sb[64:65, :], q0s[64:65, :],
                                     start=True, stop=True, tile_position=(64, 0))
                    nc.tensor.matmul(bc1[:, :], ones_sb[64:65, :], q1s[64:65, :],
                                     start=True, stop=True, tile_position=(64, 0))
                    nc.vector.tensor_mul(t0s[:], q0s[0:64, :], bc0[:])
                    nc.vector.tensor_mul(t1s[:], q1s[0:64, :], bc1[:])
                    nc.vector.reduce_sum(fs0[:], t0s[:], axis=AX.X)
                    nc.vector.reduce_sum(fs1[:], t1s[:], axis=AX.X)

                    tr0 = pss.tile([1, 64], f32, tag="tr")
                    tr1 = pss.tile([1, 64], f32, tag="tr")
                    nc.tensor.transpose(tr0[:], fs0[:], ident[0:64, 0:64])
                    nc.tensor.transpose(tr1[:], fs1[:], ident[0:64, 0:64])
                    nc.vector.tensor_copy(frow[0:1, 0:64], tr0[:])
                    nc.vector.tensor_copy(frow[0:1, 64:128], tr1[:])

                    for img in range(2):
                        seg = frow[0:1, 64 * img:64 * img + 64]
                        oseg = srow[0:1, 64 * img:64 * img + 64]
                        nc.vector.reduce_max(mx[:], seg, axis=AX.X, negate=True)
                        nc.scalar.activation(out=oseg, in_=seg, func=AF.Exp,
                                             bias=mx[:], scale=1.0)
                        nc.vector.reduce_sum(sm[:], oseg, axis=AX.X)
                        nc.vector.reciprocal(rs[:], sm[:])
                        nc.vector.tensor_scalar_mul(oseg, oseg, rs[:])

                    psc = pss.tile([128, 1], f32, tag="psc")
                    nc.tensor.transpose(psc[:], srow[:], ident[0:1, 0:1])
                    nc.vector.tensor_copy(scores[:], psc[:])

                # ---------------- pass 2: conv_v -> epilogue -> out ----------
                with (
                    tc.tile_pool(name="ps2", bufs=4, space="PSUM") as ps2,
                    tc.tile_pool(name="epi", bufs=3) as epi_p,
                    tc.tile_pool(name="gio", bufs=2) as gio,
                ):
                    MUL = mybir.AluOpType.mult
                    ADD = mybir.AluOpType.add
                    cur2 = {}
                    nblk = len(BLOCKS)

                    def gA(row, dx):
                        return A + row * WP + dx

                    def gB(row, dx):
                        return B + row * WP + dx

                    def rw(ap, nr):
                        return ap.rearrange("p (r c) -> p r c", c=W)

                    def p2_iter():
                        xt = ot = None
                        for bi, (r0, nr) in enumerate(BLOCKS):
                            g = bi // 8
                            goff = 24 * g * W
                            glen = 3840 if g < 6 else 2560
                            grp_last = (bi % 8 == 7) or bi == nblk - 1

                            pp = ps2 if (bi < 4 or bi % 2 == 0 or
                                         pool2["alt"] is None) else pool2["alt"]
                            ps = pp.tile([128, 512], f32, tag="ps",
                                         name="ps2t")
                            if bi % 8 == 0:
                                xt = gio.tile([128, 3840], bf16, tag="xt",
                                              name="xt")
                                ot = gio.tile([128, 3840], bf16, tag="ot",
                                              name="ot")

                            def mk1(first=(bi % 8 == 0), r0=r0, nr=nr, ps=ps,
                                    goff=goff, glen=glen, xt=xt):
                                if first:
                                    nc.sync.dma_start(
                                        out=xt[:, 0:glen],
                                        in_=xr_d.ap()[:, goff:goff + glen])
                                return mk_mms(y, wts["wvp1"], wts["wvs"],
                                              gA, 0, r0, nr, ps)

                            def mk2(r0=r0, nr=nr, ps=ps):
                                return mk_mms(y, wts["wvp2"], wts["wvs"],
                                              gB, 1, r0, nr, ps)

                            def epi(r0=r0, nr=nr, ps=ps, goff=goff,
                                    glen=glen, grp_last=grp_last,
                                    xt=xt, ot=ot):
                                boff = (r0 - 1) * W - goff
                                M = nr * W
                                u2 = epi_p.tile([128, 3 * W], bf16, tag="u2")
                                nc.vector.scalar_tensor_tensor(
                                    rw(u2[0:64, 0:M], nr),
                                    rw(ps[0:64, 0:M], nr), scores[0:64],
                                    r3(y[0:64, A + r0 * WP:
                                         A + (r0 + nr) * WP])[:, :, 1:1 + W],
                                    MUL, ADD)
                                u = epi_p.tile([128, 3 * W], bf16, tag="u")
                                nc.scalar.mul(u[64:128, 0:M],
                                              ps[64:128, 0:M], scores[64:128])
                                nc.vector.tensor_add(
                                    rw(u2[64:128, 0:M], nr),
                                    rw(u[64:128, 0:M], nr),
                                    r3(y[64:128, B + r0 * WP:
                                         B + (r0 + nr) * WP])[:, :, 1:1 + W])
                                rt = epi_p.tile([128, 3 * W], bf16, tag="rt")
                                nc.scalar.activation(out=rt[:, 0:M],
                                                     in_=u2[:, 0:M],
                                                     func=AF.Relu,
                                                     bias=bn2b[:],
                                                     scale=bn2s[:])
                                nc.vector.tensor_add(ot[:, boff:boff + M],
                                                     rt[:, 0:M],
                                                     xt[:, boff:boff + M])
                                if goff == 23040 and boff + M == 1440:
                                    # last group: flush in two halves so the
                                    # final DMA tail is one small transfer
                                    nc.sync.dma_start(
                                        out=out_d.ap()[:, goff:goff + 1440],
                                        in_=ot[:, 0:1440])
                                elif goff == 23040 and boff + M == 2560:
                                    nc.sync.dma_start(
                                        out=out_d.ap()[:, goff + 1440:
                                                       goff + 2560],
                                        in_=ot[:, 1440:2560])
                                elif grp_last:
                                    nc.sync.dma_start(
                                        out=out_d.ap()[:, goff:goff + glen],
                                        in_=ot[:, 0:glen])
                            yield mk1, mk2, epi

                    conv_pass(p2_iter())

        if loop:
            with tc.For_i(0, loop, 1):
                body()
        else:
            body()
        ctx_lp.__exit__(None, None, None)
    nc.compile()
    return nc


def _get_nc():
    if "nc" not in _CACHED:
        _CACHED["nc"] = _build_nc()
    return _CACHED["nc"]


def _prep_weights(w_cv1, wq, wk, wv, g1, b1, m1, v1, g2, b2, m2, v2):
    import ml_dtypes
    bf = ml_dtypes.bfloat16

    def parts(w):
        # w [cout, cin, ky, kx] -> t [cin, ky, kx, cout]
        t = np.ascontiguousarray(w.transpose(1, 2, 3, 0))
        p1 = np.concatenate([t[:, 0], t[:, 1]], axis=0)   # low=dy0, high=dy1
        p2 = np.concatenate([t[:, 1], t[:, 0]], axis=0)   # low=dy1, high=dy0
        s = np.concatenate([t[:, 2], t[:, 2]], axis=0)    # dy2 duplicated
        return (np.ascontiguousarray(p1.astype(bf)),
                np.ascontiguousarray(p2.astype(bf)),
                np.ascontiguousarray(s.astype(bf)))

    w1p1, w1p2, w1s = parts(w_cv1)
    wvp1, wvp2, wvs = parts(wv)

    scale = 1.0 / (float(W) ** 0.5 * float(H) * float(H))
    q = wq.transpose(1, 2, 3, 0).reshape(C, 9, C) * scale    # [j, 9, c]
    ks = wk.sum(axis=0).reshape(C, 9, 1)                     # [j, 9, 1]
    qa = np.concatenate([q, ks], axis=2)                     # [j, 9, 65]
    wqt = np.ascontiguousarray(np.concatenate([qa, qa], axis=0).astype(bf))

    s1 = (g1 / np.sqrt(v1 + BN_EPS)).astype(np.float32)
    b1p = (b1 - m1 * s1).astype(np.float32)
    s2 = (g2 / np.sqrt(v2 + BN_EPS)).astype(np.float32)
    b2p = (b2 - m2 * s2).astype(np.float32)

    def dup(v):
        return np.ascontiguousarray(
            np.concatenate([v, v]).reshape(128, 1).astype(np.float32))

    return dict(w1p1=w1p1, w1p2=w1p2, w1s=w1s, wvp1=wvp1, wvp2=wvp2, wvs=wvs,
                wqt=wqt, bn1s=dup(s1), bn1b=dup(b1p),
                bn2s=dup(s2), bn2b=dup(b2p))


def _stage_x(x2):
    """x2: [2, C, H, W] f32 -> (xp [128, 2*LP] bf16, xr [128, IMG] bf16)."""
    import ml_dtypes
    bf = ml_dtypes.bfloat16
    xpad = np.zeros((2, C, HP, WP), np.float32)
    xpad[:, :, 1:1 + H, 1:1 + W] = x2
    flat = xpad.reshape(2, C, PIMG)
    sh = np.zeros_like(flat)
    sh[:, :, :PIMG - WP] = flat[:, :, WP:]
    xp = np.zeros((128, 2 * LP), bf)
    xp[0:64, 0:PIMG] = flat[0]
    xp[64:128, 0:PIMG] = sh[0]
    xp[0:64, B0:B0 + PIMG] = sh[1]
    xp[64:128, B0:B0 + PIMG] = flat[1]
    xr = np.ascontiguousarray(
        x2.reshape(2, C, IMG).reshape(128, IMG).astype(bf))
    return np.ascontiguousarray(xp), xr


B0 = LP


def _ensure_axon_devices():
    """Make sure jax can see the 8 axon-tunneled NeuronCores even if the
    calling process pinned JAX_PLATFORMS=cpu before importing us."""
    import os
    envp = os.environ.get("JAX_PLATFORMS", "")
    if envp and "axon" not in envp:
        os.environ.pop("JAX_PLATFORMS", None)
    import jax
    try:
        devs = jax.devices()
        if len(devs) >= 8 and all("cpu" not in str(d).lower() for d in devs[:8]):
            return
    except Exception:
        pass
    try:
        from jax._src import xla_bridge
        xla_bridge.backends.cache_clear()
    except Exception:
        pass
    try:
        import jax.extend.backend as jeb
        jeb.clear_backends()
    except Exception:
        pass


def kernel(x, w_cv1, g1, b1, m1, v1, wq, wk, wv, g2, b2, m2, v2):
    _ensure_axon_devices()
    from concourse.bass_utils import run_bass_kernel_spmd

    x = np.asarray(x, dtype=np.float32)
    consts = _prep_weights(
        np.asarray(w_cv1, np.float32), np.asarray(wq, np.float32),
        np.asarray(wk, np.float32), np.asarray(wv, np.float32),
        np.asarray(g1, np.float32), np.asarray(b1, np.float32),
        np.asarray(m1, np.float32), np.asarray(v1, np.float32),
        np.asarray(g2, np.float32), np.asarray(b2, np.float32),
        np.asarray(m2, np.float32), np.asarray(v2, np.float32))
    nc = _get_nc()
    in_maps = []
    for i in range(8):
        xp, xr = _stage_x(x[2 * i:2 * i + 2])
        m = {"xp": xp, "xr": xr}
        m.update(consts)
        in_maps.append(m)
    res = run_bass_kernel_spmd(nc, in_maps, core_ids=list(range(8)))
    outs = [np.asarray(r["out"]).astype(np.float32).reshape(2, C, H, W)
            for r in res.results]
    return np.concatenate(outs, axis=0)


# revision 39
# speedup vs baseline: 1.0028x; 1.0002x over previous
"""Trainium2 Bass kernel for nn_Bottleneck_CSA_ConvBlock.

Computation (per image, C=64, H=W=160):
    y  = silu(bn1(conv3x3(x, w1)))
    fq = conv3x3(y, wq); fk = conv3x3(y, wk); fv = conv3x3(y, wv)
    k_sum = fk.sum(ch, h); f_scores[c] = scale * sum_hw fq[c,h,w]*k_sum[w]
    scores = softmax_c(f_scores)
    out = x + relu(bn2(scores*fv + y))

Key idea vs the plain lowering: conv contraction is only C=64 (half the
128-row PE array). Each image is stored twice in SBUF -- channels
unshifted on one partition half, shifted by one padded row (+WP) on the
other -- so a single 128-row matmul computes TWO dy-taps at once: the 9
taps of a 3x3 conv become 3 pair-matmuls plus 3 dy=2 singles (2/3 the
matmul instructions; measured ~200ns fixed cost per matmul makes count
matter as much as streamed cycles). Blocks are nr=3 rows (N=480, one
PSUM bank). The two images run on the two 64-column PE strips; emission
is software-pipelined (block b's img1 interleaves with block b-1's img2)
so the strips overlap while both images share one PSUM bank per block
(img2's group re-opens the bank with skip_group_check -- HW start only
clears has_written bits, img1's finished data is untouched).

HW constraints discovered (mini_mm.py probes): a PSUM bank's
accumulation group must keep ONE region and ONE row-position; mixing
row positions (e.g. (0,0) then (64,0)) wedges the device. Column-
disjoint same-row matmul streams overlap; diagonal ones did not.

Layout (DRAM-staged x and on-chip y):
    region A: partitions 0-63  = img1 unshifted ; 64-127 = img1 shifted(+WP)
    region B: partitions 0-63  = img2 shifted   ; 64-127 = img2 unshifted
so conv outputs land lane-aligned ([img1|img2] on psum partitions).
Pass-1 BN+SiLU runs full-width into a scratch tile; four SBUF->SBUF
DMAs distribute it to the four y quarter-layouts (unshifted + shifted),
and column sums reduce from the scratch at full width.

f_scores/k_sum fold to linear functionals of y's column sums (fq/fk
never materialized). Residual x and the output travel as bf16.

Sharding: pure data parallelism, 2 images per core across 8 cores.
"""

import numpy as np

C = 64
H = W = 160
HP = WP = 162          # padded
IMG = H * W            # 25600
PIMG = HP * WP         # 26244
LP = PIMG + 60         # region stride in the paired buffers
NCH = 26               # x chunk rows
CHH = NCH * WP + 4     # chunk region stride (+slack for dx=2 edge views)
GRP = 4                # blocks per scratch/distribute super
BLOCKS = [(1 + 3 * i, 3) for i in range(53)] + [(160, 1)]
BN_EPS = 1e-5

_CACHED = {}


def _build_nc(loop=0, act1=None):
    import concourse.bass as bass
    import concourse.tile as tile
    from concourse import bacc, mybir
    from concourse.masks import make_identity

    dt = mybir.dt
    AF = mybir.ActivationFunctionType
    AX = mybir.AxisListType
    ACT1 = AF.Silu if act1 is None else getattr(AF, act1)
    f32 = dt.float32
    bf16 = dt.bfloat16

    nc = bacc.Bacc("TRN2", target_bir_lowering=False, debug=False, num_devices=8)

    xp_d = nc.dram_tensor("xp", [128, 2 * LP], bf16, kind="ExternalInput")
    xr_d = nc.dram_tensor("xr", [128, IMG], bf16, kind="ExternalInput")
    w1p1_d = nc.dram_tensor("w1p1", [128, 3, 64], bf16, kind="ExternalInput")
    w1p2_d = nc.dram_tensor("w1p2", [128, 3, 64], bf16, kind="ExternalInput")
    w1s_d = nc.dram_tensor("w1s", [128, 3, 64], bf16, kind="ExternalInput")
    wvp1_d = nc.dram_tensor("wvp1", [128, 3, 64], bf16, kind="ExternalInput")
    wvp2_d = nc.dram_tensor("wvp2", [128, 3, 64], bf16, kind="ExternalInput")
    wvs_d = nc.dram_tensor("wvs", [128, 3, 64], bf16, kind="ExternalInput")
    wq_d = nc.dram_tensor("wqt", [128, 9, 65], bf16, kind="ExternalInput")
    bn1s_d = nc.dram_tensor("bn1s", [128, 1], f32, kind="ExternalInput")
    bn1b_d = nc.dram_tensor("bn1b", [128, 1], f32, kind="ExternalInput")
    bn2s_d = nc.dram_tensor("bn2s", [128, 1], f32, kind="ExternalInput")
    bn2b_d = nc.dram_tensor("bn2b", [128, 1], f32, kind="ExternalInput")
    out_d = nc.dram_tensor("out", [128, IMG], bf16, kind="ExternalOutput")

    A = 0
    B = LP

    def r3(ap):
        return ap.rearrange("p (r c) -> p r c", c=WP)

    with tile.TileContext(nc) as tc:
        ctx_lp = nc.allow_low_precision("bf16 matmul path; fp32 PSUM accumulation")
        ctx_lp.__enter__()

        def mk_mms(src, wpt, wst, bX, img, r0, nr, ps):
            """Thunk list: 3 pair-MMs + 3 single-MMs for one image/block.

            img=0: strip (0,0), psum [0:64], pairs low=dy0/high=dy1 at
            base row r0-1, singles via unshifted low at base r0+1.
            img=1: strip (0,64), psum [64:128], pairs (swapped weights) at
            r0-1, singles via shifted low half at base r0.
            img2's matmuls re-open the bank's accumulation group after
            img1's closed it -- legal on HW (start only clears has_written
            bits, img1's data is untouched); skip_group_check silences the
            sim's partition-blind group tracker.
            """
            po = ps[0:64] if img == 0 else ps[64:128]
            tp = (0, 0) if img == 0 else (0, 64)
            skip = img == 1
            th = []
            for dx in range(3):
                o = bX(r0 - 1, dx)
                th.append(lambda o=o, dx=dx: nc.tensor.matmul(
                    po[:, 0:nr * W], wpt[:, dx, :],
                    r3(src[0:128, o:o + nr * WP])[:, :, 0:W],
                    start=(dx == 0), stop=False, tile_position=tp,
                    skip_group_check=skip))
            srow = r0 + 1 if img == 0 else r0
            for dx in range(3):
                o = bX(srow, dx)
                th.append(lambda o=o, dx=dx: nc.tensor.matmul(
                    po[:, 0:nr * W], wst[0:64, dx, :],
                    r3(src[0:64, o:o + nr * WP])[:, :, 0:W],
                    start=False, stop=(dx == 2), tile_position=tp,
                    skip_group_check=skip))
            return th

        def conv_pass(block_iter):
            """Software-pipelined emission: block b's img1 MMs interleave
            with block b-1's img2 MMs (disjoint PE column strips overlap).
            block_iter yields (mk1, mk2, epilogue) per block."""
            prev2 = prev_epi = None
            for mk1, mk2, epi in block_iter:
                cur1 = mk1()
                if prev2 is None:
                    for f in cur1:
                        f()
                else:
                    for a, b in zip(prev2, cur1):
                        a()
                        b()
                    prev_epi()
                prev2, prev_epi = mk2(), epi
            for f in prev2:
                f()
            prev_epi()

        def body():
            with (
                tc.tile_pool(name="const", bufs=1) as const,
                tc.tile_pool(name="ybuf", bufs=1) as ybuf,
                tc.tile_pool(name="small", bufs=1) as small,
            ):
                wts = {}
                for nm, d in (("w1p1", w1p1_d), ("w1p2", w1p2_d), ("w1s", w1s_d),
                              ("wvp1", wvp1_d), ("wvp2", wvp2_d), ("wvs", wvs_d)):
                    t = const.tile([128, 3, 64], bf16, name="w_" + nm)
                    nc.scalar.dma_start(out=t[:], in_=d.ap())
                    wts[nm] = t
                wq_sb = const.tile([128, 9, 65], bf16)
                nc.scalar.dma_start(out=wq_sb[:], in_=wq_d.ap())
                bn1s = const.tile([128, 1], f32)
                nc.scalar.dma_start(out=bn1s[:], in_=bn1s_d.ap())
                bn1b = const.tile([128, 1], f32)
                nc.scalar.dma_start(out=bn1b[:], in_=bn1b_d.ap())
                bn2s = const.tile([128, 1], f32)
                nc.scalar.dma_start(out=bn2s[:], in_=bn2s_d.ap())
                bn2b = const.tile([128, 1], f32)
                nc.scalar.dma_start(out=bn2b[:], in_=bn2b_d.ap())
                ident = const.tile([128, 128], f32)
                make_identity(nc, ident[:])
                ones_sb = const.tile([128, 64], bf16)
                nc.vector.memset(ones_sb[:], 1.0)

                # persistent paired y (bf16)
                y = ybuf.tile([128, 2 * LP], bf16)
                # zero pads on the unshifted halves: top+bottom rows, l/r cols
                for p0, p1, reg in ((0, 64, A), (64, 128, B)):
                    nc.vector.memset(y[p0:p1, reg:reg + WP], 0.0)
                    nc.vector.memset(
                        y[p0:p1, reg + (H + 1) * WP:reg + (H + 2) * WP], 0.0)
                    v3 = r3(y[p0:p1, reg:reg + PIMG])
                    nc.vector.memset(v3[:, 1:H + 1, 0:1], 0.0)
                    nc.vector.memset(v3[:, 1:H + 1, WP - 1:WP], 0.0)
                # shifted halves: address row 160 = image row 161 = zeros
                for p0, p1, reg in ((64, 128, A), (0, 64, B)):
                    nc.vector.memset(y[p0:p1, reg + H * WP:reg + (H + 1) * WP], 0.0)

                C_sb = small.tile([128, WP], bf16)
                CmL = small.tile([128, WP], bf16)
                CmF = small.tile([128, WP], bf16)
                part = small.tile([128, WP], f32, tag="part")
                q0s = small.tile([65, 160], bf16)
                q1s = small.tile([65, 160], bf16)
                t0s = small.tile([64, 160], f32)
                t1s = small.tile([64, 160], f32)
                fs0 = small.tile([64, 1], f32)
                fs1 = small.tile([64, 1], f32)
                frow = small.tile([1, 128], f32)
                srow = small.tile([1, 128], f32)
                mx = small.tile([1, 1], f32, tag="mx")
                sm = small.tile([1, 1], f32, tag="sm")
                rs = small.tile([1, 1], f32, tag="rs")
                scores = small.tile([128, 1], f32)
                # pass-1 scratch: full-width ACT target, distributed to the
                # paired y regions by DMA. Col pads pre-zeroed; ACT only
                # writes interiors, so pads stay zero across reuses.
                sp0 = small.tile([128, 12 * WP], bf16, tag="sp0")
                sp1 = small.tile([128, 12 * WP], bf16, tag="sp1")
                for sp in (sp0, sp1):
                    v = sp.rearrange("p (r c) -> p r c", c=WP)
                    nc.vector.memset(v[:, :, 0:1], 0.0)
                    nc.vector.memset(v[:, :, WP - 1:WP], 0.0)

                xpap = xp_d.ap()

                # ---------------- pass 1: conv1 -> y (paired), column sums ----
                with (
                    tc.tile_pool(name="chunks", bufs=2) as chunks,
                    tc.tile_pool(name="ps1", bufs=8, space="PSUM") as ps1,
                ):
                    cur = {}
                    nblk = len(BLOCKS)

                    def p1_iter():
                        for bi, (r0, nr) in enumerate(BLOCKS):
                            k = bi // 8
                            si = bi // GRP
                            sp = (sp0, sp1)[si % 2]
                            sup_r0 = BLOCKS[si * GRP][0]
                            sup_last = (bi % GRP == GRP - 1) or bi == nblk - 1

                            ps = ps1.tile([128, 512], f32, tag="ps",
                                          name="ps1t")

                            def mk1(k=k, first=(bi % 8 == 0), r0=r0, nr=nr,
                                    ps=ps):
                                if first:
                                    ir0 = 24 * k
                                    nir = 26 if k < 6 else 18
                                    ch = chunks.tile([128, 2 * CHH], bf16,
                                                     tag="ch", name="cht")
                                    pieces = ((0, 5), (5, 9), (14, nir - 14)) \
                                        if k == 0 else ((0, nir),)
                                    for po, pn in pieces:
                                        nc.sync.dma_start(
                                            out=ch[:, po * WP:(po + pn) * WP],
                                            in_=xpap[:, A + (ir0 + po) * WP:
                                                     A + (ir0 + po + pn) * WP])
                                        nc.sync.dma_start(
                                            out=ch[:, CHH + po * WP:
                                                    CHH + (po + pn) * WP],
                                            in_=xpap[:, B + (ir0 + po) * WP:
                                                     B + (ir0 + po + pn) * WP])
                                    cur["ch"], cur["ir0"] = ch, ir0
                                ch, ir0 = cur["ch"], cur["ir0"]

                                def bA(row, dx):
                                    return (row - ir0) * WP + dx
                                return mk_mms(ch, wts["w1p1"], wts["w1s"],
                                              bA, 0, r0, nr, ps)

                            def mk2(r0=r0, nr=nr, ps=ps):
                                ch, ir0 = cur["ch"], cur["ir0"]

                                def bB(row, dx):
                                    return CHH + (row - ir0) * WP + dx
                                return mk_mms(ch, wts["w1p2"], wts["w1s"],
                                              bB, 1, r0, nr, ps)

                            def epi(bi=bi, r0=r0, nr=nr, ps=ps, sp=sp,
                                    sup_r0=sup_r0, sup_last=sup_last, si=si):
                                loff = (r0 - sup_r0) * WP
                                nc.scalar.activation(
                                    out=r3(sp[:, loff:loff + nr * WP])[:, :, 1:1 + W],
                                    in_=ps[0:128, 0:nr * W],
                                    func=ACT1, bias=bn1b[:], scale=bn1s[:])
                                if sup_last:
                                    srows = (r0 + nr) - sup_r0
                                    sl = srows * WP
                                    nc.sync.dma_start(
                                        out=y[0:64, A + sup_r0 * WP:
                                              A + sup_r0 * WP + sl],
                                        in_=sp[0:64, 0:sl])
                                    nc.sync.dma_start(
                                        out=y[64:128, B + sup_r0 * WP:
                                              B + sup_r0 * WP + sl],
                                        in_=sp[64:128, 0:sl])
                                    nc.gpsimd.dma_start(
                                        out=y[64:128, A + (sup_r0 - 1) * WP:
                                              A + (sup_r0 - 1) * WP + sl],
                                        in_=sp[0:64, 0:sl])
                                    nc.gpsimd.dma_start(
                                        out=y[0:64, B + (sup_r0 - 1) * WP:
                                              B + (sup_r0 - 1) * WP + sl],
                                        in_=sp[64:128, 0:sl])
                                    nc.vector.reduce_sum(
                                        part[:],
                                        r3(sp[:, 0:sl])
                                        .rearrange("p r c -> p c r"),
                                        axis=AX.X)
                                    if si == 0:
                                        nc.vector.tensor_copy(C_sb[:], part[:])
                                    else:
                                        nc.vector.tensor_add(C_sb[:], C_sb[:],
                                                             part[:])
                            yield mk1, mk2, epi

                    conv_pass(p1_iter())

                # ---------------- scores (small path) ----------------
                with tc.tile_pool(name="pss", bufs=2, space="PSUM") as pss:
                    nc.vector.tensor_sub(CmL[0:64, :], C_sb[0:64, :],
                                         y[0:64, A + H * WP:A + (H + 1) * WP])
                    nc.vector.tensor_sub(CmL[64:128, :], C_sb[64:128, :],
                                         y[64:128, B + H * WP:B + (H + 1) * WP])
                    nc.vector.tensor_sub(CmF[0:64, :], C_sb[0:64, :],
                                         y[0:64, A + WP:A + 2 * WP])
                    nc.vector.tensor_sub(CmF[64:128, :], C_sb[64:128, :],
                                         y[64:128, B + WP:B + 2 * WP])
                    s_of = {0: CmL, 1: C_sb, 2: CmF}

                    qp0 = pss.tile([65, 160], f32, tag="qp")
                    qp1 = pss.tile([65, 160], f32, tag="qp")
                    for k9 in range(9):
                        dy, dx = divmod(k9, 3)
                        src = s_of[dy]
                        nc.tensor.matmul(
                            qp0[:, :], wq_sb[0:64, k9, :], src[0:64, dx:dx + 160],
                            start=(k9 == 0), stop=(k9 == 8), tile_position=(0, 0))
                    for k9 in range(9):
                        dy, dx = divmod(k9, 3)
                        src = s_of[dy]
                        nc.tensor.matmul(
                            qp1[:, :], wq_sb[64:128, k9, :], src[64:128, dx:dx + 160],
                            start=(k9 == 0), stop=(k9 == 8), tile_position=(64, 0))
                    nc.vector.tensor_copy(q0s[:], qp0[:])
                    nc.vector.tensor_copy(q1s[:], qp1[:])

                    # broadcast k_sum row (partition 64) across 64 partitions
                    bc0 = pss.tile([64, 160], f32, tag="bc")
                    bc1 = pss.tile([64, 160], f32, tag="bc")
                    nc.tensor.matmul(bc0[:, :], ones_sb[64:65, :], q0s[64:65, :],
                                     start=True, stop=True, tile_position=(64, 0))
                    nc.tensor.matmul(bc1[:, :], ones_sb[64:65, :], q1s[64:65, :],
                                     start=True, stop=True, tile_position=(64, 0))
                    nc.vector.tensor_mul(t0s[:], q0s[0:64, :], bc0[:])
                    nc.vector.tensor_mul(t1s[:], q1s[0:64, :], bc1[:])
                    nc.vector.reduce_sum(fs0[:], t0s[:], axis=AX.X)
                    nc.vector.reduce_sum(fs1[:], t1s[:], axis=AX.X)

                    tr0 = pss.tile([1, 64], f32, tag="tr")
                    tr1 = pss.tile([1, 64], f32, tag="tr")
                    nc.tensor.transpose(tr0[:], fs0[:], ident[0:64, 0:64])
                    nc.tensor.transpose(tr1[:], fs1[:], ident[0:64, 0:64])
                    nc.vector.tensor_copy(frow[0:1, 0:64], tr0[:])
                    nc.vector.tensor_copy(frow[0:1, 64:128], tr1[:])

                    for img in range(2):
                        seg = frow[0:1, 64 * img:64 * img + 64]
                        oseg = srow[0:1, 64 * img:64 * img + 64]
                        nc.vector.reduce_max(mx[:], seg, axis=AX.X, negate=True)
                        nc.scalar.activation(out=oseg, in_=seg, func=AF.Exp,
                                             bias=mx[:], scale=1.0)
                        nc.vector.reduce_sum(sm[:], oseg, axis=AX.X)
                        nc.vector.reciprocal(rs[:], sm[:])
                        nc.vector.tensor_scalar_mul(oseg, oseg, rs[:])

                    psc = pss.tile([128, 1], f32, tag="psc")
                    nc.tensor.transpose(psc[:], srow[:], ident[0:1, 0:1])
                    nc.vector.tensor_copy(scores[:], psc[:])

                # ---------------- pass 2: conv_v -> epilogue -> out ----------
                with (
                    tc.tile_pool(name="ps2", bufs=4, space="PSUM") as ps2,
                    tc.tile_pool(name="epi", bufs=3) as epi_p,
                    tc.tile_pool(name="gio", bufs=2) as gio,
                ):
                    MUL = mybir.AluOpType.mult
                    ADD = mybir.AluOpType.add
                    cur2 = {}
                    nblk = len(BLOCKS)

                    def gA(row, dx):
                        return A + row * WP + dx

                    def gB(row, dx):
                        return B + row * WP + dx

                    def rw(ap, nr):
                        return ap.rearrange("p (r c) -> p r c", c=W)

                    def p2_iter():
                        xt = ot = None
                        for bi, (r0, nr) in enumerate(BLOCKS):
                            g = bi // 8
                            goff = 24 * g * W
                            glen = 3840 if g < 6 else 2560
                            grp_last = (bi % 8 == 7) or bi == nblk - 1

                            pp = ps2 if (bi < 4 or bi % 2 == 0 or
                                         pool2["alt"] is None) else pool2["alt"]
                            ps = pp.tile([128, 512], f32, tag="ps",
                                         name="ps2t")
                            if bi % 8 == 0:
                                xt = gio.tile([128, 3840], bf16, tag="xt",
                                              name="xt")
                                ot = gio.tile([128, 3840], bf16, tag="ot",
                                              name="ot")

                            def mk1(first=(bi % 8 == 0), r0=r0, nr=nr, ps=ps,
                                    goff=goff, glen=glen, xt=xt):
                                if first:
                                    nc.scalar.dma_start(
                                        out=xt[:, 0:glen],
                                        in_=xr_d.ap()[:, goff:goff + glen])
                                return mk_mms(y, wts["wvp1"], wts["wvs"],
                                              gA, 0, r0, nr, ps)

                            def mk2(r0=r0, nr=nr, ps=ps):
                                return mk_mms(y, wts["wvp2"], wts["wvs"],
                                              gB, 1, r0, nr, ps)

                            def epi(r0=r0, nr=nr, ps=ps, goff=goff,
                                    glen=glen, grp_last=grp_last,
                                    xt=xt, ot=ot):
                                boff = (r0 - 1) * W - goff
                                M = nr * W
                                u2 = epi_p.tile([128, 3 * W], bf16, tag="u2")
                                nc.vector.scalar_tensor_tensor(
                                    rw(u2[0:64, 0:M], nr),
                                    rw(ps[0:64, 0:M], nr), scores[0:64],
                                    r3(y[0:64, A + r0 * WP:
                                         A + (r0 + nr) * WP])[:, :, 1:1 + W],
                                    MUL, ADD)
                                u = epi_p.tile([128, 3 * W], bf16, tag="u")
                                nc.scalar.mul(u[64:128, 0:M],
                                              ps[64:128, 0:M], scores[64:128])
                                nc.vector.tensor_add(
                                    rw(u2[64:128, 0:M], nr),
                                    rw(u[64:128, 0:M], nr),
                                    r3(y[64:128, B + r0 * WP:
                                         B + (r0 + nr) * WP])[:, :, 1:1 + W])
                                rt = epi_p.tile([128, 3 * W], bf16, tag="rt")
                                nc.scalar.activation(out=rt[:, 0:M],
                                                     in_=u2[:, 0:M],
                                                     func=AF.Relu,
                                                     bias=bn2b[:],
                                                     scale=bn2s[:])
                                nc.vector.tensor_add(ot[:, boff:boff + M],
                                                     rt[:, 0:M],
                                                     xt[:, boff:boff + M])
                                if goff == 23040 and boff + M == 1440:
                                    # last group: flush in two halves so the
                                    # final DMA tail is one small transfer
                                    nc.sync.dma_start(
                                        out=out_d.ap()[:, goff:goff + 1440],
                                        in_=ot[:, 0:1440])
                                elif goff == 23040 and boff + M == 2560:
                                    nc.sync.dma_start(
                                        out=out_d.ap()[:, goff + 1440:
                                                       goff + 2560],
                                        in_=ot[:, 1440:2560])
                                elif grp_last:
                                    nc.sync.dma_start(
                                        out=out_d.ap()[:, goff:goff + glen],
                                        in_=ot[:, 0:glen])
                            yield mk1, mk2, epi

                    conv_pass(p2_iter())

        if loop:
            with tc.For_i(0, loop, 1):
                body()
        else:
            body()
        ctx_lp.__exit__(None, None, None)
    nc.compile()
    return nc


def _get_nc():
    if "nc" not in _CACHED:
        _CACHED["nc"] = _build_nc()
    return _CACHED["nc"]


def _prep_weights(w_cv1, wq, wk, wv, g1, b1, m1, v1, g2, b2, m2, v2):
    import ml_dtypes
    bf = ml_dtypes.bfloat16

    def parts(w):
        # w [cout, cin, ky, kx] -> t [cin, ky, kx, cout]
        t = np.ascontiguousarray(w.transpose(1, 2, 3, 0))
        p1 = np.concatenate([t[:, 0], t[:, 1]], axis=0)   # low=dy0, high=dy1
        p2 = np.concatenate([t[:, 1], t[:, 0]], axis=0)   # low=dy1, high=dy0
        s = np.concatenate([t[:, 2], t[:, 2]], axis=0)    # dy2 duplicated
        return (np.ascontiguousarray(p1.astype(bf)),
                np.ascontiguousarray(p2.astype(bf)),
                np.ascontiguousarray(s.astype(bf)))

    w1p1, w1p2, w1s = parts(w_cv1)
    wvp1, wvp2, wvs = parts(wv)

    scale = 1.0 / (float(W) ** 0.5 * float(H) * float(H))
    q = wq.transpose(1, 2, 3, 0).reshape(C, 9, C) * scale    # [j, 9, c]
    ks = wk.sum(axis=0).reshape(C, 9, 1)                     # [j, 9, 1]
    qa = np.concatenate([q, ks], axis=2)                     # [j, 9, 65]
    wqt = np.ascontiguousarray(np.concatenate([qa, qa], axis=0).astype(bf))

    s1 = (g1 / np.sqrt(v1 + BN_EPS)).astype(np.float32)
    b1p = (b1 - m1 * s1).astype(np.float32)
    s2 = (g2 / np.sqrt(v2 + BN_EPS)).astype(np.float32)
    b2p = (b2 - m2 * s2).astype(np.float32)

    def dup(v):
        return np.ascontiguousarray(
            np.concatenate([v, v]).reshape(128, 1).astype(np.float32))

    return dict(w1p1=w1p1, w1p2=w1p2, w1s=w1s, wvp1=wvp1, wvp2=wvp2, wvs=wvs,
                wqt=wqt, bn1s=dup(s1), bn1b=dup(b1p),
                bn2s=dup(s2), bn2b=dup(b2p))


def _stage_x(x2):
    """x2: [2, C, H, W] f32 -> (xp [128, 2*LP] bf16, xr [128, IMG] bf16)."""
    import ml_dtypes
    bf = ml_dtypes.bfloat16
    xpad = np.zeros((2, C, HP, WP), np.float32)
    xpad[:, :, 1:1 + H, 1:1 + W] = x2
    flat = xpad.reshape(2, C, PIMG)
    sh = np.zeros_like(flat)
    sh[:, :, :PIMG - WP] = flat[:, :, WP:]
    xp = np.zeros((128, 2 * LP), bf)
    xp[0:64, 0:PIMG] = flat[0]
    xp[64:128, 0:PIMG] = sh[0]
    xp[0:64, B0:B0 + PIMG] = sh[1]
    xp[64:128, B0:B0 + PIMG] = flat[1]
    xr = np.ascontiguousarray(
        x2.reshape(2, C, IMG).reshape(128, IMG).astype(bf))
    return np.ascontiguousarray(xp), xr


B0 = LP


def _ensure_axon_devices():
    """Make sure jax can see the 8 axon-tunneled NeuronCores even if the
    calling process pinned JAX_PLATFORMS=cpu before importing us."""
    import os
    envp = os.environ.get("JAX_PLATFORMS", "")
    if envp and "axon" not in envp:
        os.environ.pop("JAX_PLATFORMS", None)
    import jax
    try:
        devs = jax.devices()
        if len(devs) >= 8 and all("cpu" not in str(d).lower() for d in devs[:8]):
            return
    except Exception:
        pass
    try:
        from jax._src import xla_bridge
        xla_bridge.backends.cache_clear()
    except Exception:
        pass
    try:
        import jax.extend.backend as jeb
        jeb.clear_backends()
    except Exception:
        pass


def kernel(x, w_cv1, g1, b1, m1, v1, wq, wk, wv, g2, b2, m2, v2):
    _ensure_axon_devices()
    from concourse.bass_utils import run_bass_kernel_spmd

    x = np.asarray(x, dtype=np.float32)
    consts = _prep_weights(
        np.asarray(w_cv1, np.float32), np.asarray(wq, np.float32),
        np.asarray(wk, np.float32), np.asarray(wv, np.float32),
        np.asarray(g1, np.float32), np.asarray(b1, np.float32),
        np.asarray(m1, np.float32), np.asarray(v1, np.float32),
        np.asarray(g2, np.float32), np.asarray(b2, np.float32),
        np.asarray(m2, np.float32), np.asarray(v2, np.float32))
    nc = _get_nc()
    in_maps = []
    for i in range(8):
        xp, xr = _stage_x(x[2 * i:2 * i + 2])
        m = {"xp": xp, "xr": xr}
        m.update(consts)
        in_maps.append(m)
    res = run_bass_kernel_spmd(nc, in_maps, core_ids=list(range(8)))
    outs = [np.asarray(r["out"]).astype(np.float32).reshape(2, C, H, W)
            for r in res.results]
    return np.concatenate(outs, axis=0)


# revision 42
# speedup vs baseline: 1.0040x; 1.0012x over previous
"""Trainium2 Bass kernel for nn_Bottleneck_CSA_ConvBlock.

Computation (per image, C=64, H=W=160):
    y  = silu(bn1(conv3x3(x, w1)))
    fq = conv3x3(y, wq); fk = conv3x3(y, wk); fv = conv3x3(y, wv)
    k_sum = fk.sum(ch, h); f_scores[c] = scale * sum_hw fq[c,h,w]*k_sum[w]
    scores = softmax_c(f_scores)
    out = x + relu(bn2(scores*fv + y))

Key idea vs the plain lowering: conv contraction is only C=64 (half the
128-row PE array). Each image is stored twice in SBUF -- channels
unshifted on one partition half, shifted by one padded row (+WP) on the
other -- so a single 128-row matmul computes TWO dy-taps at once: the 9
taps of a 3x3 conv become 3 pair-matmuls plus 3 dy=2 singles (2/3 the
matmul instructions; measured ~200ns fixed cost per matmul makes count
matter as much as streamed cycles). Blocks are nr=3 rows (N=480, one
PSUM bank). The two images run on the two 64-column PE strips; emission
is software-pipelined (block b's img1 interleaves with block b-1's img2)
so the strips overlap while both images share one PSUM bank per block
(img2's group re-opens the bank with skip_group_check -- HW start only
clears has_written bits, img1's finished data is untouched).

HW constraints discovered (mini_mm.py probes): a PSUM bank's
accumulation group must keep ONE region and ONE row-position; mixing
row positions (e.g. (0,0) then (64,0)) wedges the device. Column-
disjoint same-row matmul streams overlap; diagonal ones did not.

Layout (DRAM-staged x and on-chip y):
    region A: partitions 0-63  = img1 unshifted ; 64-127 = img1 shifted(+WP)
    region B: partitions 0-63  = img2 shifted   ; 64-127 = img2 unshifted
so conv outputs land lane-aligned ([img1|img2] on psum partitions).
Pass-1 BN+SiLU runs full-width into a scratch tile; four SBUF->SBUF
DMAs distribute it to the four y quarter-layouts (unshifted + shifted),
and column sums reduce from the scratch at full width.

f_scores/k_sum fold to linear functionals of y's column sums (fq/fk
never materialized). Residual x and the output travel as bf16.

Sharding: pure data parallelism, 2 images per core across 8 cores.
"""

import numpy as np

C = 64
H = W = 160
HP = WP = 162          # padded
IMG = H * W            # 25600
PIMG = HP * WP         # 26244
LP = PIMG + 60         # region stride in the paired buffers
NCH = 26               # x chunk rows
CHH = NCH * WP + 4     # chunk region stride (+slack for dx=2 edge views)
GRP = 4                # blocks per scratch/distribute super
BLOCKS = [(1 + 3 * i, 3) for i in range(53)] + [(160, 1)]
BN_EPS = 1e-5

_CACHED = {}


def _build_nc(loop=0, act1=None):
    import concourse.bass as bass
    import concourse.tile as tile
    from concourse import bacc, mybir
    from concourse.masks import make_identity

    dt = mybir.dt
    AF = mybir.ActivationFunctionType
    AX = mybir.AxisListType
    ACT1 = AF.Silu if act1 is None else getattr(AF, act1)
    f32 = dt.float32
    bf16 = dt.bfloat16

    nc = bacc.Bacc("TRN2", target_bir_lowering=False, debug=False, num_devices=8)

    xp_d = nc.dram_tensor("xp", [128, 2 * LP], bf16, kind="ExternalInput")
    xr_d = nc.dram_tensor("xr", [128, IMG], bf16, kind="ExternalInput")
    w1p1_d = nc.dram_tensor("w1p1", [128, 3, 64], bf16, kind="ExternalInput")
    w1p2_d = nc.dram_tensor("w1p2", [128, 3, 64], bf16, kind="ExternalInput")
    w1s_d = nc.dram_tensor("w1s", [128, 3, 64], bf16, kind="ExternalInput")
    wvp1_d = nc.dram_tensor("wvp1", [128, 3, 64], bf16, kind="ExternalInput")
    wvp2_d = nc.dram_tensor("wvp2", [128, 3, 64], bf16, kind="ExternalInput")
    wvs_d = nc.dram_tensor("wvs", [128, 3, 64], bf16, kind="ExternalInput")
    wq_d = nc.dram_tensor("wqt", [128, 9, 65], bf16, kind="ExternalInput")
    bn1s_d = nc.dram_tensor("bn1s", [128, 1], f32, kind="ExternalInput")
    bn1b_d = nc.dram_tensor("bn1b", [128, 1], f32, kind="ExternalInput")
    bn2s_d = nc.dram_tensor("bn2s", [128, 1], f32, kind="ExternalInput")
    bn2b_d = nc.dram_tensor("bn2b", [128, 1], f32, kind="ExternalInput")
    out_d = nc.dram_tensor("out", [128, IMG], bf16, kind="ExternalOutput")

    A = 0
    B = LP

    def r3(ap):
        return ap.rearrange("p (r c) -> p r c", c=WP)

    with tile.TileContext(nc) as tc:
        ctx_lp = nc.allow_low_precision("bf16 matmul path; fp32 PSUM accumulation")
        ctx_lp.__enter__()

        def mk_mms(src, wpt, wst, bX, img, r0, nr, ps):
            """Thunk list: 3 pair-MMs + 3 single-MMs for one image/block.

            img=0: strip (0,0), psum [0:64], pairs low=dy0/high=dy1 at
            base row r0-1, singles via unshifted low at base r0+1.
            img=1: strip (0,64), psum [64:128], pairs (swapped weights) at
            r0-1, singles via shifted low half at base r0.
            img2's matmuls re-open the bank's accumulation group after
            img1's closed it -- legal on HW (start only clears has_written
            bits, img1's data is untouched); skip_group_check silences the
            sim's partition-blind group tracker.
            """
            po = ps[0:64] if img == 0 else ps[64:128]
            tp = (0, 0) if img == 0 else (0, 64)
            skip = img == 1
            th = []
            for dx in range(3):
                o = bX(r0 - 1, dx)
                th.append(lambda o=o, dx=dx: nc.tensor.matmul(
                    po[:, 0:nr * W], wpt[:, dx, :],
                    r3(src[0:128, o:o + nr * WP])[:, :, 0:W],
                    start=(dx == 0), stop=False, tile_position=tp,
                    skip_group_check=skip))
            srow = r0 + 1 if img == 0 else r0
            for dx in range(3):
                o = bX(srow, dx)
                th.append(lambda o=o, dx=dx: nc.tensor.matmul(
                    po[:, 0:nr * W], wst[0:64, dx, :],
                    r3(src[0:64, o:o + nr * WP])[:, :, 0:W],
                    start=False, stop=(dx == 2), tile_position=tp,
                    skip_group_check=skip))
            return th

        def conv_pass(block_iter):
            """Software-pipelined emission: block b's img1 MMs interleave
            with block b-1's img2 MMs (disjoint PE column strips overlap).
            block_iter yields (mk1, mk2, epilogue) per block."""
            prev2 = prev_epi = None
            for mk1, mk2, epi in block_iter:
                cur1 = mk1()
                if prev2 is None:
                    for f in cur1:
                        f()
                else:
                    for a, b in zip(prev2, cur1):
                        a()
                        b()
                    prev_epi()
                prev2, prev_epi = mk2(), epi
            for f in prev2:
                f()
            prev_epi()

        def body():
            with (
                tc.tile_pool(name="const", bufs=1) as const,
                tc.tile_pool(name="ybuf", bufs=1) as ybuf,
                tc.tile_pool(name="small", bufs=1) as small,
            ):
                wts = {}
                for nm, d in (("w1p1", w1p1_d), ("w1p2", w1p2_d), ("w1s", w1s_d),
                              ("wvp1", wvp1_d), ("wvp2", wvp2_d), ("wvs", wvs_d)):
                    t = const.tile([128, 3, 64], bf16, name="w_" + nm)
                    nc.scalar.dma_start(out=t[:], in_=d.ap())
                    wts[nm] = t
                wq_sb = const.tile([128, 9, 65], bf16)
                nc.scalar.dma_start(out=wq_sb[:], in_=wq_d.ap())
                bn1s = const.tile([128, 1], f32)
                nc.scalar.dma_start(out=bn1s[:], in_=bn1s_d.ap())
                bn1b = const.tile([128, 1], f32)
                nc.scalar.dma_start(out=bn1b[:], in_=bn1b_d.ap())
                bn2s = const.tile([128, 1], f32)
                nc.scalar.dma_start(out=bn2s[:], in_=bn2s_d.ap())
                bn2b = const.tile([128, 1], f32)
                nc.scalar.dma_start(out=bn2b[:], in_=bn2b_d.ap())
                ident = const.tile([128, 128], f32)
                make_identity(nc, ident[:])
                ones_sb = const.tile([128, 64], bf16)
                nc.vector.memset(ones_sb[:], 1.0)

                # persistent paired y (bf16)
                y = ybuf.tile([128, 2 * LP], bf16)
                # zero pads on the unshifted halves: top+bottom rows, l/r cols
                for p0, p1, reg in ((0, 64, A), (64, 128, B)):
                    nc.vector.memset(y[p0:p1, reg:reg + WP], 0.0)
                    nc.vector.memset(
                        y[p0:p1, reg + (H + 1) * WP:reg + (H + 2) * WP], 0.0)
                    v3 = r3(y[p0:p1, reg:reg + PIMG])
                    nc.vector.memset(v3[:, 1:H + 1, 0:1], 0.0)
                    nc.vector.memset(v3[:, 1:H + 1, WP - 1:WP], 0.0)
                # shifted halves: address row 160 = image row 161 = zeros
                for p0, p1, reg in ((64, 128, A), (0, 64, B)):
                    nc.vector.memset(y[p0:p1, reg + H * WP:reg + (H + 1) * WP], 0.0)

                C_sb = small.tile([128, WP], bf16)
                CmL = small.tile([128, WP], bf16)
                CmF = small.tile([128, WP], bf16)
                part = small.tile([128, WP], f32, tag="part")
                q0s = small.tile([65, 160], bf16)
                q1s = small.tile([65, 160], bf16)
                t0s = small.tile([64, 160], f32)
                t1s = small.tile([64, 160], f32)
                fs0 = small.tile([64, 1], f32)
                fs1 = small.tile([64, 1], f32)
                frow = small.tile([1, 128], f32)
                srow = small.tile([1, 128], f32)
                mx = small.tile([1, 1], f32, tag="mx")
                sm = small.tile([1, 1], f32, tag="sm")
                rs = small.tile([1, 1], f32, tag="rs")
                scores = small.tile([128, 1], f32)
                # pass-1 scratch: full-width ACT target, distributed to the
                # paired y regions by DMA. Col pads pre-zeroed; ACT only
                # writes interiors, so pads stay zero across reuses.
                sp0 = small.tile([128, 12 * WP], bf16, tag="sp0")
                sp1 = small.tile([128, 12 * WP], bf16, tag="sp1")
                for sp in (sp0, sp1):
                    v = sp.rearrange("p (r c) -> p r c", c=WP)
                    nc.vector.memset(v[:, :, 0:1], 0.0)
                    nc.vector.memset(v[:, :, WP - 1:WP], 0.0)

                xpap = xp_d.ap()

                # ---------------- pass 1: conv1 -> y (paired), column sums ----
                with (
                    tc.tile_pool(name="chunks", bufs=2) as chunks,
                    tc.tile_pool(name="ps1", bufs=8, space="PSUM") as ps1,
                ):
                    cur = {}
                    nblk = len(BLOCKS)

                    def p1_iter():
                        for bi, (r0, nr) in enumerate(BLOCKS):
                            k = bi // 8
                            si = bi // GRP
                            sp = (sp0, sp1)[si % 2]
                            sup_r0 = BLOCKS[si * GRP][0]
                            sup_last = (bi % GRP == GRP - 1) or bi == nblk - 1

                            ps = ps1.tile([128, 512], f32, tag="ps",
                                          name="ps1t")

                            def mk1(k=k, first=(bi % 8 == 0), r0=r0, nr=nr,
                                    ps=ps):
                                if first:
                                    ir0 = 24 * k
                                    nir = 26 if k < 6 else 18
                                    ch = chunks.tile([128, 2 * CHH], bf16,
                                                     tag="ch", name="cht")
                                    pieces = ((0, 5), (5, 9), (14, nir - 14)) \
                                        if k == 0 else ((0, nir),)
                                    for po, pn in pieces:
                                        nc.sync.dma_start(
                                            out=ch[:, po * WP:(po + pn) * WP],
                                            in_=xpap[:, A + (ir0 + po) * WP:
                                                     A + (ir0 + po + pn) * WP])
                                        nc.sync.dma_start(
                                            out=ch[:, CHH + po * WP:
                                                    CHH + (po + pn) * WP],
                                            in_=xpap[:, B + (ir0 + po) * WP:
                                                     B + (ir0 + po + pn) * WP])
                                    cur["ch"], cur["ir0"] = ch, ir0
                                ch, ir0 = cur["ch"], cur["ir0"]

                                def bA(row, dx):
                                    return (row - ir0) * WP + dx
                                return mk_mms(ch, wts["w1p1"], wts["w1s"],
                                              bA, 0, r0, nr, ps)

                            def mk2(r0=r0, nr=nr, ps=ps):
                                ch, ir0 = cur["ch"], cur["ir0"]

                                def bB(row, dx):
                                    return CHH + (row - ir0) * WP + dx
                                return mk_mms(ch, wts["w1p2"], wts["w1s"],
                                              bB, 1, r0, nr, ps)

                            def epi(bi=bi, r0=r0, nr=nr, ps=ps, sp=sp,
                                    sup_r0=sup_r0, sup_last=sup_last, si=si):
                                loff = (r0 - sup_r0) * WP
                                nc.scalar.activation(
                                    out=r3(sp[:, loff:loff + nr * WP])[:, :, 1:1 + W],
                                    in_=ps[0:128, 0:nr * W],
                                    func=ACT1, bias=bn1b[:], scale=bn1s[:])
                                if sup_last:
                                    srows = (r0 + nr) - sup_r0
                                    sl = srows * WP
                                    nc.sync.dma_start(
                                        out=y[0:64, A + sup_r0 * WP:
                                              A + sup_r0 * WP + sl],
                                        in_=sp[0:64, 0:sl])
                                    nc.sync.dma_start(
                                        out=y[64:128, B + sup_r0 * WP:
                                              B + sup_r0 * WP + sl],
                                        in_=sp[64:128, 0:sl])
                                    nc.gpsimd.dma_start(
                                        out=y[64:128, A + (sup_r0 - 1) * WP:
                                              A + (sup_r0 - 1) * WP + sl],
                                        in_=sp[0:64, 0:sl])
                                    nc.gpsimd.dma_start(
                                        out=y[0:64, B + (sup_r0 - 1) * WP:
                                              B + (sup_r0 - 1) * WP + sl],
                                        in_=sp[64:128, 0:sl])
                                    nc.vector.reduce_sum(
                                        part[:],
                                        r3(sp[:, 0:sl])
                                        .rearrange("p r c -> p c r"),
                                        axis=AX.X)
                                    if si == 0:
                                        nc.vector.tensor_copy(C_sb[:], part[:])
                                    else:
                                        nc.vector.tensor_add(C_sb[:], C_sb[:],
                                                             part[:])
                            yield mk1, mk2, epi

                    conv_pass(p1_iter())

                # ---------------- scores (small path) ----------------
                with tc.tile_pool(name="pss", bufs=2, space="PSUM") as pss:
                    nc.vector.tensor_sub(CmL[0:64, :], C_sb[0:64, :],
                                         y[0:64, A + H * WP:A + (H + 1) * WP])
                    nc.vector.tensor_sub(CmL[64:128, :], C_sb[64:128, :],
                                         y[64:128, B + H * WP:B + (H + 1) * WP])
                    nc.vector.tensor_sub(CmF[0:64, :], C_sb[0:64, :],
                                         y[0:64, A + WP:A + 2 * WP])
                    nc.vector.tensor_sub(CmF[64:128, :], C_sb[64:128, :],
                                         y[64:128, B + WP:B + 2 * WP])
                    s_of = {0: CmL, 1: C_sb, 2: CmF}

                    qp0 = pss.tile([65, 160], f32, tag="qp")
                    qp1 = pss.tile([65, 160], f32, tag="qp")
                    for k9 in range(9):
                        dy, dx = divmod(k9, 3)
                        src = s_of[dy]
                        nc.tensor.matmul(
                            qp0[:, :], wq_sb[0:64, k9, :], src[0:64, dx:dx + 160],
                            start=(k9 == 0), stop=(k9 == 8), tile_position=(0, 0))
                    for k9 in range(9):
                        dy, dx = divmod(k9, 3)
                        src = s_of[dy]
                        nc.tensor.matmul(
                            qp1[:, :], wq_sb[64:128, k9, :], src[64:128, dx:dx + 160],
                            start=(k9 == 0), stop=(k9 == 8), tile_position=(64, 0))
                    nc.vector.tensor_copy(q0s[:], qp0[:])
                    nc.vector.tensor_copy(q1s[:], qp1[:])

                    # broadcast k_sum row (partition 64) across 64 partitions
                    bc0 = pss.tile([64, 160], f32, tag="bc")
                    bc1 = pss.tile([64, 160], f32, tag="bc")
                    nc.tensor.matmul(bc0[:, :], ones_sb[64:65, :], q0s[64:65, :],
                                     start=True, stop=True, tile_position=(64, 0))
                    nc.tensor.matmul(bc1[:, :], ones_sb[64:65, :], q1s[64:65, :],
                                     start=True, stop=True, tile_position=(64, 0))
                    nc.vector.tensor_mul(t0s[:], q0s[0:64, :], bc0[:])
                    nc.vector.tensor_mul(t1s[:], q1s[0:64, :], bc1[:])
                    nc.vector.reduce_sum(fs0[:], t0s[:], axis=AX.X)
                    nc.vector.reduce_sum(fs1[:], t1s[:], axis=AX.X)

                    tr0 = pss.tile([1, 64], f32, tag="tr")
                    tr1 = pss.tile([1, 64], f32, tag="tr")
                    nc.tensor.transpose(tr0[:], fs0[:], ident[0:64, 0:64])
                    nc.tensor.transpose(tr1[:], fs1[:], ident[0:64, 0:64])
                    nc.vector.tensor_copy(frow[0:1, 0:64], tr0[:])
                    nc.vector.tensor_copy(frow[0:1, 64:128], tr1[:])

                    for img in range(2):
                        seg = frow[0:1, 64 * img:64 * img + 64]
                        oseg = srow[0:1, 64 * img:64 * img + 64]
                        nc.vector.reduce_max(mx[:], seg, axis=AX.X, negate=True)
                        nc.scalar.activation(out=oseg, in_=seg, func=AF.Exp,
                                             bias=mx[:], scale=1.0)
                        nc.vector.reduce_sum(sm[:], oseg, axis=AX.X)
                        nc.vector.reciprocal(rs[:], sm[:])
                        nc.vector.tensor_scalar_mul(oseg, oseg, rs[:])

                    psc = pss.tile([128, 1], f32, tag="psc")
                    nc.tensor.transpose(psc[:], srow[:], ident[0:1, 0:1])
                    nc.vector.tensor_copy(scores[:], psc[:])

                # ---------------- pass 2: conv_v -> epilogue -> out ----------
                with (
                    tc.tile_pool(name="ps2", bufs=4, space="PSUM") as ps2,
                    tc.tile_pool(name="epi", bufs=3) as epi_p,
                    tc.tile_pool(name="gio", bufs=2) as gio,
                ):
                    MUL = mybir.AluOpType.mult
                    ADD = mybir.AluOpType.add
                    cur2 = {}
                    nblk = len(BLOCKS)

                    def gA(row, dx):
                        return A + row * WP + dx

                    def gB(row, dx):
                        return B + row * WP + dx

                    def rw(ap, nr):
                        return ap.rearrange("p (r c) -> p r c", c=W)

                    def p2_iter():
                        xt = ot = None
                        for bi, (r0, nr) in enumerate(BLOCKS):
                            g = bi // 8
                            goff = 24 * g * W
                            glen = 3840 if g < 6 else 2560
                            grp_last = (bi % 8 == 7) or bi == nblk - 1

                            pp = ps2 if (bi < 4 or bi % 2 == 0 or
                                         pool2["alt"] is None) else pool2["alt"]
                            ps = pp.tile([128, 512], f32, tag="ps",
                                         name="ps2t")
                            if bi % 8 == 0:
                                xt = gio.tile([128, 3840], bf16, tag="xt",
                                              name="xt")
                                ot = gio.tile([128, 3840], bf16, tag="ot",
                                              name="ot")

                            def mk1(first=(bi % 8 == 0), r0=r0, nr=nr, ps=ps,
                                    goff=goff, glen=glen, xt=xt):
                                if first:
                                    nc.scalar.dma_start(
                                        out=xt[:, 0:glen],
                                        in_=xr_d.ap()[:, goff:goff + glen])
                                return mk_mms(y, wts["wvp1"], wts["wvs"],
                                              gA, 0, r0, nr, ps)

                            def mk2(r0=r0, nr=nr, ps=ps):
                                return mk_mms(y, wts["wvp2"], wts["wvs"],
                                              gB, 1, r0, nr, ps)

                            def epi(r0=r0, nr=nr, ps=ps, goff=goff,
                                    glen=glen, grp_last=grp_last,
                                    xt=xt, ot=ot):
                                boff = (r0 - 1) * W - goff
                                M = nr * W
                                u2 = epi_p.tile([128, 3 * W], bf16, tag="u2")
                                nc.vector.scalar_tensor_tensor(
                                    rw(u2[0:64, 0:M], nr),
                                    rw(ps[0:64, 0:M], nr), scores[0:64],
                                    r3(y[0:64, A + r0 * WP:
                                         A + (r0 + nr) * WP])[:, :, 1:1 + W],
                                    MUL, ADD)
                                u = epi_p.tile([128, 3 * W], bf16, tag="u")
                                nc.scalar.mul(u[64:128, 0:M],
                                              ps[64:128, 0:M], scores[64:128])
                                nc.vector.tensor_add(
                                    rw(u2[64:128, 0:M], nr),
                                    rw(u[64:128, 0:M], nr),
                                    r3(y[64:128, B + r0 * WP:
                                         B + (r0 + nr) * WP])[:, :, 1:1 + W])
                                rt = epi_p.tile([128, 3 * W], bf16, tag="rt")
                                nc.scalar.activation(out=rt[:, 0:M],
                                                     in_=u2[:, 0:M],
                                                     func=AF.Relu,
                                                     bias=bn2b[:],
                                                     scale=bn2s[:])
                                nc.vector.tensor_add(ot[:, boff:boff + M],
                                                     rt[:, 0:M],
                                                     xt[:, boff:boff + M])
                                if goff == 23040 and boff + M == 1920:
                                    # last group: flush in two halves so the
                                    # final DMA tail is one small transfer
                                    nc.sync.dma_start(
                                        out=out_d.ap()[:, goff:goff + 1920],
                                        in_=ot[:, 0:1920])
                                elif goff == 23040 and boff + M == 2560:
                                    nc.sync.dma_start(
                                        out=out_d.ap()[:, goff + 1920:
                                                       goff + 2560],
                                        in_=ot[:, 1920:2560])
                                elif grp_last:
                                    nc.sync.dma_start(
                                        out=out_d.ap()[:, goff:goff + glen],
                                        in_=ot[:, 0:glen])
                            yield mk1, mk2, epi

                    conv_pass(p2_iter())

        if loop:
            with tc.For_i(0, loop, 1):
                body()
        else:
            body()
        ctx_lp.__exit__(None, None, None)
    nc.compile()
    return nc


def _get_nc():
    if "nc" not in _CACHED:
        _CACHED["nc"] = _build_nc()
    return _CACHED["nc"]


def _prep_weights(w_cv1, wq, wk, wv, g1, b1, m1, v1, g2, b2, m2, v2):
    import ml_dtypes
    bf = ml_dtypes.bfloat16

    def parts(w):
        # w [cout, cin, ky, kx] -> t [cin, ky, kx, cout]
        t = np.ascontiguousarray(w.transpose(1, 2, 3, 0))
        p1 = np.concatenate([t[:, 0], t[:, 1]], axis=0)   # low=dy0, high=dy1
        p2 = np.concatenate([t[:, 1], t[:, 0]], axis=0)   # low=dy1, high=dy0
        s = np.concatenate([t[:, 2], t[:, 2]], axis=0)    # dy2 duplicated
        return (np.ascontiguousarray(p1.astype(bf)),
                np.ascontiguousarray(p2.astype(bf)),
                np.ascontiguousarray(s.astype(bf)))

    w1p1, w1p2, w1s = parts(w_cv1)
    wvp1, wvp2, wvs = parts(wv)

    scale = 1.0 / (float(W) ** 0.5 * float(H) * float(H))
    q = wq.transpose(1, 2, 3, 0).reshape(C, 9, C) * scale    # [j, 9, c]
    ks = wk.sum(axis=0).reshape(C, 9, 1)                     # [j, 9, 1]
    qa = np.concatenate([q, ks], axis=2)                     # [j, 9, 65]
    wqt = np.ascontiguousarray(np.concatenate([qa, qa], axis=0).astype(bf))

    s1 = (g1 / np.sqrt(v1 + BN_EPS)).astype(np.float32)
    b1p = (b1 - m1 * s1).astype(np.float32)
    s2 = (g2 / np.sqrt(v2 + BN_EPS)).astype(np.float32)
    b2p = (b2 - m2 * s2).astype(np.float32)

    def dup(v):
        return np.ascontiguousarray(
            np.concatenate([v, v]).reshape(128, 1).astype(np.float32))

    return dict(w1p1=w1p1, w1p2=w1p2, w1s=w1s, wvp1=wvp1, wvp2=wvp2, wvs=wvs,
                wqt=wqt, bn1s=dup(s1), bn1b=dup(b1p),
                bn2s=dup(s2), bn2b=dup(b2p))


def _stage_x(x2):
    """x2: [2, C, H, W] f32 -> (xp [128, 2*LP] bf16, xr [128, IMG] bf16)."""
    import ml_dtypes
    bf = ml_dtypes.bfloat16
    xpad = np.zeros((2, C, HP, WP), np.float32)
    xpad[:, :, 1:1 + H, 1:1 + W] = x2
    flat = xpad.reshape(2, C, PIMG)
    sh = np.zeros_like(flat)
    sh[:, :, :PIMG - WP] = flat[:, :, WP:]
    xp = np.zeros((128, 2 * LP), bf)
    xp[0:64, 0:PIMG] = flat[0]
    xp[64:128, 0:PIMG] = sh[0]
    xp[0:64, B0:B0 + PIMG] = sh[1]
    xp[64:128, B0:B0 + PIMG] = flat[1]
    xr = np.ascontiguousarray(
        x2.reshape(2, C, IMG).reshape(128, IMG).astype(bf))
    return np.ascontiguousarray(xp), xr


B0 = LP


def _ensure_axon_devices():
    """Make sure jax can see the 8 axon-tunneled NeuronCores even if the
    calling process pinned JAX_PLATFORMS=cpu before importing us."""
    import os
    envp = os.environ.get("JAX_PLATFORMS", "")
    if envp and "axon" not in envp:
        os.environ.pop("JAX_PLATFORMS", None)
    import jax
    try:
        devs = jax.devices()
        if len(devs) >= 8 and all("cpu" not in str(d).lower() for d in devs[:8]):
            return
    except Exception:
        pass
    try:
        from jax._src import xla_bridge
        xla_bridge.backends.cache_clear()
    except Exception:
        pass
    try:
        import jax.extend.backend as jeb
        jeb.clear_backends()
    except Exception:
        pass


def kernel(x, w_cv1, g1, b1, m1, v1, wq, wk, wv, g2, b2, m2, v2):
    _ensure_axon_devices()
    from concourse.bass_utils import run_bass_kernel_spmd

    x = np.asarray(x, dtype=np.float32)
    consts = _prep_weights(
        np.asarray(w_cv1, np.float32), np.asarray(wq, np.float32),
        np.asarray(wk, np.float32), np.asarray(wv, np.float32),
        np.asarray(g1, np.float32), np.asarray(b1, np.float32),
        np.asarray(m1, np.float32), np.asarray(v1, np.float32),
        np.asarray(g2, np.float32), np.asarray(b2, np.float32),
        np.asarray(m2, np.float32), np.asarray(v2, np.float32))
    nc = _get_nc()
    in_maps = []
    for i in range(8):
        xp, xr = _stage_x(x[2 * i:2 * i + 2])
        m = {"xp": xp, "xr": xr}
        m.update(consts)
        in_maps.append(m)
    res = run_bass_kernel_spmd(nc, in_maps, core_ids=list(range(8)))
    outs = [np.asarray(r["out"]).astype(np.float32).reshape(2, C, H, W)
            for r in res.results]
    return np.concatenate(outs, axis=0)
